# revision 32
# baseline (speedup 1.0000x reference)
"""GPT forward pass on 8 Trainium2 NeuronCores — v2 (head-sharded attention).

Trunk (LN/FFN/residual/lm_head) is token-parallel: core c owns token tile c
(128 tokens) of each of the 4 sequences. Attention is Megatron head-sharded:
core c owns global heads {2c, 2c+1} and computes full causal attention for
those heads over all 4096 tokens — every (head, seq) unit has the identical
causal block structure (q-tile qt needs qt+1 k-tiles), so the SPMD program
is uniform across cores while skipping all fully-masked blocks (36/64).

Per layer: LN1 -> AllGather h^T (1 MB payload) -> Q/K/V for my 2 heads over
all tokens (SBUF-resident, no DRAM round-trip) -> causal attention ->
row-parallel Wo partial -> bf16 ReduceScatter back to token owners ->
residual -> LN2 -> FFN (token-parallel). lm_head is token-sharded: each
core streams the full lnf-folded Wlm and emits bf16 logits for its 512
tokens; blm is added on the host.

LN gains are folded into consuming weights host-side (exact); ln2/lnf
biases into b1/blm (exact); projection bias b2 enters the PSUM via a K=1
ones-row matmul; bo is added after the ReduceScatter. All matmuls bf16
with fp32 PSUM; residual stream and softmax stats stay fp32. Softmax is
transposed-layout with denominators from a ones-column of V (no max
subtraction needed at these scales — matches the reference to ~5e-3).
"""

import os
import sys

for _p in ("/opt/trn_rl_repo",):
    if os.path.isdir(_p) and _p not in sys.path:
        sys.path.insert(0, _p)

import numpy as np
import ml_dtypes

BF16NP = ml_dtypes.bfloat16

import concourse.bass as bass
import concourse.mybir as mybir
import concourse.tile as tile
from concourse import bacc
from concourse.bass_utils import run_bass_kernel_spmd
from concourse.masks import make_identity

F32 = mybir.dt.float32
BF = mybir.dt.bfloat16
AF = mybir.ActivationFunctionType

V, C, T, H, L, B = 32000, 1024, 1024, 16, 4, 4
HD = C // H          # 64
FF = 4 * C           # 4096
NCORES = 8
TL = 512             # local tokens per core (4 seqs x 128)
TT = B * T // 1      # 4096 total tokens (seq-major: t = s*1024 + pos)
SEQ = B
NT = TL // 128       # 4 local t-tiles; tile tt = seq tt
NCT = C // 128       # 8 c-tiles
NFT = FF // 128      # 32 f-tiles
NTB = TT // 128      # 32 global t-blocks
NCH = 64             # vocab chunks
VCW = V // NCH       # 500
LN_EPS = 1e-5

_prog_cache = {}


def _ap(t, offset, pattern):
    return bass.AP(tensor=t.tensor if isinstance(t, bass.AP) else t, offset=offset, ap=pattern)


def _build(LL=L, sim=False):
    key = (LL, sim)
    if key in _prog_cache:
        return _prog_cache[key]

    nc = bacc.Bacc("TRN2", target_bir_lowering=False, debug=False, num_devices=NCORES)

    x0 = nc.dram_tensor("x0", [TL, C], F32, kind="ExternalInput")
    mask_d = nc.dram_tensor("maskd", [128, 128], BF, kind="ExternalInput")
    wq_d = nc.dram_tensor("wq", [L, C, 128], BF, kind="ExternalInput")   # my head cols
    wk_d = nc.dram_tensor("wk", [L, C, 128], BF, kind="ExternalInput")
    wv_d = nc.dram_tensor("wv", [L, C, 128], BF, kind="ExternalInput")
    wo_d = nc.dram_tensor("wo", [L, C, C], BF, kind="ExternalInput")
    w1_d = nc.dram_tensor("w1", [L, C, FF], BF, kind="ExternalInput")
    w2_d = nc.dram_tensor("w2", [L, FF, C], BF, kind="ExternalInput")
    b1_d = nc.dram_tensor("b1", [L, FF], F32, kind="ExternalInput")
    bo_d = nc.dram_tensor("bo", [L, C], BF, kind="ExternalInput")
    b2_d = nc.dram_tensor("b2", [L, C], BF, kind="ExternalInput")
    wlm_d = nc.dram_tensor("wlm", [NCH, NCT, 128, VCW], BF, kind="ExternalInput")

    logits_d = nc.dram_tensor("logits", [TL, V], BF, kind="ExternalOutput")

    HTSZ = C * TL            # elems in one core's hT payload (1 MB bf16)

    with tile.TileContext(nc) as tc:
        import contextlib

        with contextlib.ExitStack() as ctx:
            # SBUF pools (~per-partition KB)
            const = ctx.enter_context(tc.tile_pool(name="const", bufs=1))      # .6
            xpool = ctx.enter_context(tc.tile_pool(name="x", bufs=1))          # 16
            hpool = ctx.enter_context(tc.tile_pool(name="h", bufs=5))          # 10
            tpool = ctx.enter_context(tc.tile_pool(name="hT", bufs=1))         # 8
            big = ctx.enter_context(tc.tile_pool(name="big", bufs=2))          # 64
            qkt = ctx.enter_context(tc.tile_pool(name="qkt", bufs=1))          # 16
            vsb_p = ctx.enter_context(tc.tile_pool(name="vsb", bufs=1))        # 8.3
            otm_p = ctx.enter_context(tc.tile_pool(name="otm", bufs=1))        # 8
            w4 = ctx.enter_context(tc.tile_pool(name="w4", bufs=4))            # 16
            qwp = ctx.enter_context(tc.tile_pool(name="qw", bufs=1))           # 6
            gbpool = ctx.enter_context(tc.tile_pool(name="gb", bufs=1))        # 4.5
            misc = ctx.enter_context(tc.tile_pool(name="misc", bufs=2))        # .6
            pt_pool = ctx.enter_context(tc.tile_pool(name="pt", bufs=3))       # 6
            oraw_pool = ctx.enter_context(tc.tile_pool(name="oraw", bufs=2))   # 8
            os_pool = ctx.enter_context(tc.tile_pool(name="oS", bufs=2))       # 4
            rcp_pool = ctx.enter_context(tc.tile_pool(name="rcp", bufs=2))     # 8
            rb_pool = ctx.enter_context(tc.tile_pool(name="rb", bufs=2))       # 8
            pd_pool = ctx.enter_context(tc.tile_pool(name="pd", bufs=3))       # 6
            rcv_pool = ctx.enter_context(tc.tile_pool(name="rcv", bufs=2))     # 4
            lgout = ctx.enter_context(tc.tile_pool(name="lgout", bufs=2))      # 8
            ps_acc = ctx.enter_context(tc.tile_pool(name="psacc", bufs=4, space="PSUM"))
            ps_st = ctx.enter_context(tc.tile_pool(name="psst", bufs=2, space="PSUM"))
            ps_ov = ctx.enter_context(tc.tile_pool(name="psov", bufs=2, space="PSUM"))
            dram = ctx.enter_context(tc.tile_pool(name="dram", bufs=1, space="DRAM"))

            ident = const.tile([128, 128], BF, name="ident")
            make_identity(nc, ident)
            eps_t = const.tile([128, 1], F32, name="eps")
            nc.vector.memset(eps_t[:], LN_EPS)
            mask_t = const.tile([128, 128], BF, name="mask")
            nc.sync.dma_start(out=mask_t[:], in_=mask_d[:])
            ones1 = const.tile([1, 128], BF, name="ones1")
            nc.vector.memset(ones1[:], 1.0)

            hT_loc = dram.tile([HTSZ], BF, name="hT_loc")
            o_loc = dram.tile([NCORES * 128 * 512], BF, name="o_loc")

            # persistent residual stream fp32: tile tt = seq tt
            x_t = [xpool.tile([128, C], F32, tag=f"x{tt}", name=f"x{tt}") for tt in range(NT)]
            for tt in range(NT):
                nc.sync.dma_start(out=x_t[tt][:], in_=x0[tt * 128:(tt + 1) * 128, :])

            def emit_ln():
                """x_t -> (x-m)*rstd bf16, transposed hT tiles (g/b folded away)."""
                h_tiles = []
                for tt in range(NT):
                    stats = misc.tile([128, 2, 6], F32, name="stats", tag="stats")
                    xv = x_t[tt][:].rearrange("p (s d) -> p s d", s=2)
                    nc.vector.bn_stats(out=stats[:, 0, :], in_=xv[:, 0, :])
                    nc.vector.bn_stats(out=stats[:, 1, :], in_=xv[:, 1, :])
                    mv = misc.tile([128, 2], F32, name="mv", tag="mv")
                    nc.vector.bn_aggr(out=mv[:], in_=stats[:])
                    rstd = misc.tile([128, 1], F32, name="rstd", tag="rstd")
                    nc.scalar.activation(rstd[:], mv[:, 1:2], AF.Sqrt, bias=eps_t[:])
                    nc.vector.reciprocal(rstd[:], rstd[:])
                    h = hpool.tile([128, C], BF, tag="h", name="h")
                    nc.vector.tensor_scalar(
                        out=h[:], in0=x_t[tt][:], scalar1=mv[:, 0:1], scalar2=rstd[:],
                        op0=mybir.AluOpType.subtract, op1=mybir.AluOpType.mult,
                    )
                    h_tiles.append(h)
                hT_all = tpool.tile([128, NCT, 512], BF, tag="hTall", name="hTall")
                for ct in range(NCT):
                    pst = ps_st.tile([128, 512], BF, tag="st", name="pst")
                    for tt in range(NT):
                        nc.tensor.transpose(
                            pst[:, tt * 128:(tt + 1) * 128],
                            h_tiles[tt][:, ct * 128:(ct + 1) * 128],
                            ident[:],
                        )
                    nc.vector.tensor_copy(out=hT_all[:, ct, :], in_=pst[:])
                return hT_all

            for l in range(LL):
                lw = l % L
                # per-layer bias/const tiles (issued early; Pool queue quiet)
                b2_r = gbpool.tile([1, C], BF, tag="b2r", name="b2_r")
                nc.sync.dma_start(out=b2_r[:], in_=b2_d[lw:lw + 1, :])
                bo_r = gbpool.tile([1, C], BF, tag="bor", name="bo_r")
                nc.sync.dma_start(out=bo_r[:], in_=bo_d[lw:lw + 1, :])
                b1_t = misc.tile([128, NFT], F32, tag="b1", name="b1_t")
                nc.gpsimd.dma_start(
                    out=b1_t[:], in_=_ap(b1_d, lw * FF, [[1, 128], [128, NFT]])
                )
                # my-head projection weights [128c-in-ct, ct, 128d] — one DMA each
                wq_t = qwp.tile([128, NCT, 128], BF, tag="qw", name="wq_t")
                wk_t = qwp.tile([128, NCT, 128], BF, tag="kw", name="wk_t")
                wv_t = qwp.tile([128, NCT, 128], BF, tag="vw", name="wv_t")
                for wt, wd in ((wq_t, wq_d), (wk_t, wk_d), (wv_t, wv_d)):
                    nc.sync.dma_start(
                        out=wt[:],
                        in_=_ap(wd[0:1, 0:1, 0:1], lw * C * 128,
                                [[128, 128], [128 * 128, NCT], [1, 128]]),
                    )

                # ---- LN1 -> hT, publish (one DMA), AllGather ----
                hT = emit_ln()
                nc.sync.dma_start(
                    out=_ap(hT_loc, 0, [[512, 128], [128 * 512, NCT], [1, 512]]),
                    in_=hT[:],
                )
                hT_full = dram.tile(
                    [NCORES * HTSZ], BF,
                    addr_space="Local" if sim else "Shared", name=f"hT_full{l}",
                )
                if sim:
                    nc.sync.dma_start(
                        out=_ap(hT_full, 0, [[2048, HTSZ // 2048], [1, 2048]]),
                        in_=_ap(hT_loc, 0, [[2048, HTSZ // 2048], [1, 2048]]),
                    )
                else:
                    nc.gpsimd.collective_compute(
                        "AllGather",
                        mybir.AluOpType.bypass,
                        replica_groups=[list(range(NCORES))],
                        ins=[_ap(hT_loc, 0, [[2048, HTSZ // 2048], [1, 2048]])],
                        outs=[_ap(hT_full, 0, [[2048, NCORES * HTSZ // 2048], [1, 2048]])],
                    )

                # gathered h^T lands per rank-chunk so QKV(seq s) starts after
                # 2 chunks, and attention(s) overlaps QKV(s+1)
                hT_sb = [None, None]

                def load_chunk(ch):
                    half = ch // 4
                    if hT_sb[half] is None:
                        hT_sb[half] = big.tile(
                            [128, 4, NCT, 512], BF, tag="big", name=f"hTsb{half}"
                        )
                    nc.sync.dma_start(
                        out=hT_sb[half][:, ch % 4, :, :],
                        in_=_ap(hT_full, ch * HTSZ, [[512, 128], [128 * 512, NCT], [1, 512]]),
                    )

                def htf(ct, ch):
                    return hT_sb[ch // 4][:, ch % 4, ct, :]

                qT_s, kT_s, v_ss, oTm_s = [], [], [], []
                for s in range(SEQ):
                    qT_s.append(qkt.tile([128, 1024], BF, tag=f"qT{s}", name=f"qT{s}"))
                    kT_s.append(qkt.tile([128, 1024], BF, tag=f"kT{s}", name=f"kT{s}"))
                    v_ss.append(vsb_p.tile([128, 8, 2, HD + 1], BF, tag=f"v{s}", name=f"v{s}"))
                    oTm_s.append(otm_p.tile([128, 1024], BF, tag=f"oTm{s}", name=f"oTm{s}"))

                for s in range(SEQ):
                    for hh in range(2):
                        load_chunk(2 * s + hh)
                    # K^T then Q^T for this seq (2 chunks each)
                    for dst, wt_l in ((kT_s[s], wk_t), (qT_s[s], wq_t)):
                        for hh in range(2):
                            ch = 2 * s + hh
                            ps = ps_acc.tile([128, 512], F32, tag="acc", name="acc")
                            for ct in range(NCT):
                                nc.tensor.matmul(
                                    ps[:], wt_l[:, ct, :], htf(ct, ch),
                                    start=(ct == 0), stop=(ct == NCT - 1),
                                )
                            nc.vector.tensor_copy(
                                out=dst[:, hh * 512:(hh + 1) * 512], in_=ps[:]
                            )
                    # V natural [t, my 128 d] with ones column
                    v_sb = v_ss[s]
                    nc.vector.memset(v_sb[:, :, :, HD:HD + 1], 1.0)
                    for i in range(8):
                        tb = s * 8 + i
                        ps = ps_acc.tile([128, 128], F32, tag="acc", name="psv")
                        for ct in range(NCT):
                            nc.tensor.matmul(
                                ps[:], htf(ct, tb // 4)[:, (tb % 4) * 128:(tb % 4 + 1) * 128],
                                wv_t[:, ct, :],
                                start=(ct == 0), stop=(ct == NCT - 1),
                            )
                        nc.vector.tensor_copy(out=v_sb[:, i, 0, 0:HD], in_=ps[:, 0:HD])
                        nc.vector.tensor_copy(out=v_sb[:, i, 1, 0:HD], in_=ps[:, HD:2 * HD])

                    # ---- causal attention for this seq, both heads ----
                    oraw = [
                        oraw_pool.tile([HD + 1, 1024], F32, tag="oraw", name="oraw")
                        for _ in range(2)
                    ]
                    for qt in range(8):
                        kept = qt + 1
                        for hp in range(2):
                            poff = hp * HD
                            q_sl = qT_s[s][poff:poff + HD, qt * 128:(qt + 1) * 128]
                            pT = pt_pool.tile([128, 8, 128], BF, tag="pt", name="pt")
                            for half in range((kept + 3) // 4):
                                cnt = min(4, kept - half * 4)
                                st = ps_st.tile([128, 4, 128], F32, tag="st", name="st")
                                for k4 in range(cnt):
                                    kt = half * 4 + k4
                                    nc.tensor.matmul(
                                        st[:, k4, :],
                                        kT_s[s][poff:poff + HD, kt * 128:(kt + 1) * 128],
                                        q_sl, start=True, stop=True,
                                    )
                                nc.scalar.activation(
                                    pT[:, half * 4:half * 4 + cnt, :], st[:, 0:cnt, :], AF.Exp
                                )
                            nc.vector.tensor_mul(
                                out=pT[:, qt, :], in0=pT[:, qt, :], in1=mask_t[:]
                            )
                            ov = ps_ov.tile([128, 128], F32, tag="ov", name="ov")
                            for i in range(kept):
                                nc.tensor.matmul(
                                    ov[0:HD + 1, :], v_sb[:, i, hp, :], pT[:, i, :],
                                    start=(i == 0), stop=(i == kept - 1),
                                )
                            nc.vector.tensor_copy(
                                out=oraw[hp][:, qt * 128:(qt + 1) * 128], in_=ov[0:HD + 1, :]
                            )
                    for hp in range(2):
                        recips = rcp_pool.tile([1, 1024], F32, tag="recips", name="recips")
                        nc.vector.reciprocal(recips[:], oraw[hp][HD:HD + 1, :])
                        rc_b = dram.tile([1024], F32, name=f"rcb{l}_{s}_{hp}")
                        nc.sync.dma_start(out=rc_b[:], in_=recips[:])
                        rb = rb_pool.tile([HD, 1024], F32, tag="rb", name="rb")
                        nc.gpsimd.dma_start(out=rb[:], in_=_ap(rc_b, 0, [[0, HD], [1, 1024]]))
                        if hp == 0:
                            nc.gpsimd.tensor_mul(
                                out=oTm_s[s][0:HD, :], in0=oraw[hp][0:HD, :], in1=rb[:]
                            )
                        else:
                            oS = os_pool.tile([HD, 1024], BF, tag="oS", name="oS")
                            nc.gpsimd.tensor_mul(out=oS[:], in0=oraw[hp][0:HD, :], in1=rb[:])
                            nc.sync.dma_start(out=oTm_s[s][HD:128, :], in_=oS[:])
                    nc.sync.dma_start(
                        out=_ap(o_loc, 2 * s * 128 * 512, [[512, 128], [128 * 512, 2], [1, 512]]),
                        in_=oTm_s[s][:],
                    )
                o_recv = dram.tile([NCORES * 128 * 512], BF, name=f"orecv{l}")
                if sim:
                    nc.sync.dma_start(
                        out=_ap(o_recv, 0, [[2048, NCORES * 128 * 512 // 2048], [1, 2048]]),
                        in_=_ap(o_loc, 0, [[2048, NCORES * 128 * 512 // 2048], [1, 2048]]),
                    )
                else:
                    nc.gpsimd.collective_compute(
                        "AllToAll",
                        mybir.AluOpType.bypass,
                        replica_groups=[list(range(NCORES))],
                        ins=[_ap(o_loc, 0, [[2048, NCORES * 128 * 512 // 2048], [1, 2048]])],
                        outs=[_ap(o_recv, 0, [[2048, NCORES * 128 * 512 // 2048], [1, 2048]])],
                    )
                orv = otm_p.tile([128, NCT, 512], BF, tag="orv", name="orv")
                nc.sync.dma_start(
                    out=orv[:],
                    in_=_ap(o_recv, 0, [[512, 128], [128 * 512, NCT], [1, 512]]),
                )
                for nf in range(2):
                    pss = [ps_acc.tile([128, 512], F32, tag="acc", name="acc") for _ in range(4)]
                    for cg in range(2):
                        wt = w4.tile([128, 4, 512], BF, tag="w", name="wot")
                        nc.sync.dma_start(
                            out=wt[:],
                            in_=_ap(wo_d[0:1, 0:1, 0:1],
                                    lw * C * C + cg * 512 * C + nf * 512,
                                    [[C, 128], [128 * C, 4], [1, 512]]),
                        )
                        for ci in range(4):
                            ct = cg * 4 + ci
                            for tt in range(NT):
                                nc.tensor.matmul(
                                    pss[tt][:], orv[:, ct, tt * 128:(tt + 1) * 128], wt[:, ci, :],
                                    start=(ct == 0), stop=False,
                                )
                    for tt in range(NT):
                        nc.tensor.matmul(
                            pss[tt][:], ones1[:], bo_r[:, nf * 512:(nf + 1) * 512],
                            start=False, stop=True,
                        )
                        xs = x_t[tt][:, nf * 512:(nf + 1) * 512]
                        nc.vector.tensor_add(out=xs, in0=xs, in1=pss[tt][:])

                # ---- FFN (token-parallel, biases in-psum / in-activation) ----
                h2T = emit_ln()
                ug = big.tile([128, NFT, 512], BF, tag="big", name="ug")
                for fg in range(8):
                    pss = [ps_acc.tile([128, 512], F32, tag="acc", name="acc") for _ in range(4)]
                    for hf in range(2):
                        wt = w4.tile([128, 4, 512], BF, tag="w", name="w1t")
                        nc.sync.dma_start(
                            out=wt[:],
                            in_=_ap(w1_d[0:1, 0:1, 0:1],
                                    lw * C * FF + hf * 512 * FF + fg * 512,
                                    [[FF, 128], [128 * FF, 4], [1, 512]]),
                        )
                        for ci in range(4):
                            ct = hf * 4 + ci
                            for f4 in range(4):
                                nc.tensor.matmul(
                                    pss[f4][:], wt[:, ci, f4 * 128:(f4 + 1) * 128], h2T[:, ct, :],
                                    start=(ct == 0), stop=(ct == NCT - 1),
                                )
                    for f4 in range(4):
                        ft = fg * 4 + f4
                        nc.scalar.activation(
                            ug[:, ft, :], pss[f4][:], AF.Gelu, bias=b1_t[:, ft:ft + 1]
                        )
                for nf in range(2):
                    pss = [ps_acc.tile([128, 512], F32, tag="acc", name="acc") for _ in range(4)]
                    for g8 in range(8):
                        wt = w4.tile([128, 4, 512], BF, tag="w", name="w2t")
                        nc.sync.dma_start(
                            out=wt[:],
                            in_=_ap(w2_d[0:1, 0:1, 0:1],
                                    lw * FF * C + g8 * 512 * C + nf * 512,
                                    [[C, 128], [128 * C, 4], [1, 512]]),
                        )
                        for fi in range(4):
                            ft = g8 * 4 + fi
                            for tt in range(NT):
                                nc.tensor.matmul(
                                    pss[tt][:], ug[:, ft, tt * 128:(tt + 1) * 128], wt[:, fi, :],
                                    start=(ft == 0), stop=False,
                                )
                    for tt in range(NT):
                        nc.tensor.matmul(
                            pss[tt][:], ones1[:], b2_r[:, nf * 512:(nf + 1) * 512],
                            start=False, stop=True,
                        )
                        xs = x_t[tt][:, nf * 512:(nf + 1) * 512]
                        nc.vector.tensor_add(out=xs, in0=xs, in1=pss[tt][:])

            # ---- final LN (folded) + token-sharded lm_head ----
            hfT = emit_ln()
            for ch in range(NCH):
                wlm_c = big.tile([128, NCT, VCW], BF, tag="big", name="wlm_c")
                nc.sync.dma_start(
                    out=wlm_c[:],
                    in_=_ap(wlm_d[0:1, 0:1, 0:1, 0:1], ch * NCT * 128 * VCW,
                            [[VCW, 128], [128 * VCW, NCT], [1, VCW]]),
                )
                for th in range(2):
                    lg = lgout.tile([128, 2, VCW], BF, tag="lg", name="lg")
                    for ti in range(2):
                        tt = th * 2 + ti
                        ps = ps_acc.tile([128, VCW], F32, tag="acc", name="acc")
                        for ct in range(NCT):
                            nc.tensor.matmul(
                                ps[:], hfT[:, ct, tt * 128:(tt + 1) * 128], wlm_c[:, ct, :],
                                start=(ct == 0), stop=(ct == NCT - 1),
                            )
                        if ti % 2 == 0:
                            nc.vector.tensor_copy(out=lg[:, ti, :], in_=ps[:])
                        else:
                            nc.scalar.activation(lg[:, ti, :], ps[:], AF.Copy)
                    nc.sync.dma_start(
                        out=_ap(logits_d[0:1, 0:1], th * 2 * 128 * V + ch * VCW,
                                [[V, 128], [128 * V, 2], [1, VCW]]),
                        in_=lg[:],
                    )

    nc.compile()
    _prog_cache[key] = nc
    return nc


def _prep_inputs(inputs):
    f = {k: np.asarray(v) for k, v in inputs.items()}
    idx = f["idx"].astype(np.int64)
    emb = f["emb"].astype(np.float32)
    pos = f["pos_enc"].astype(np.float32)
    x_full = emb[idx] + pos[None, :, :]          # [B,T,C] f32

    scale = HD ** -0.5
    g1 = f["ln1_g"].astype(np.float32)
    b1ln = f["ln1_b"].astype(np.float32)
    g2 = f["ln2_g"].astype(np.float32)
    b2ln = f["ln2_b"].astype(np.float32)
    gf = f["lnf_g"].astype(np.float32)
    bfln = f["lnf_b"].astype(np.float32)
    W1 = f["W1"].astype(np.float32)
    Wv = f["Wv"].astype(np.float32)
    Wo = f["Wo"].astype(np.float32)
    Wlm = f["Wlm"].astype(np.float32)

    bf = lambda a: np.ascontiguousarray(a, dtype=np.float32).astype(BF16NP)
    b1_f = f["b1"].astype(np.float32) + np.einsum("lc,lcf->lf", b2ln, W1)
    bo_f = f["bo"].astype(np.float32) + np.einsum(
        "ld,ldc->lc", np.einsum("lc,lcd->ld", b1ln, Wv), Wo
    )
    blm_f = f["blm"].astype(np.float32) + bfln @ Wlm

    wq_s = f["Wq"].astype(np.float32) * scale * g1[:, :, None]
    wk_s = f["Wk"].astype(np.float32) * g1[:, :, None]
    wv_s = Wv * g1[:, :, None]
    wlm_blocks = np.ascontiguousarray(
        (Wlm * gf[:, None]).reshape(NCT, 128, NCH, VCW).transpose(2, 0, 1, 3)
    ).astype(BF16NP)

    shared = {
        "w1": bf(W1 * g2[:, :, None]),
        "w2": bf(f["W2"]),
        "b1": b1_f.astype(np.float32),
        "bo": bf(bo_f),
        "wo": bf(Wo),
        "b2": bf(f["b2"]),
        "wlm": wlm_blocks,
        "maskd": np.triu(np.ones((128, 128), dtype=np.float32)).astype(BF16NP),
    }

    x_flat = np.ascontiguousarray(x_full.reshape(B * T, C), dtype=np.float32)
    in_maps = []
    for c in range(NCORES):
        hc = slice(c * 128, (c + 1) * 128)
        im = dict(shared)
        # core c owns flat tokens [c*512, (c+1)*512) — seq c//2, half c%2 —
        # so gathered-hT chunk r is exactly rank r's contiguous token block
        im["x0"] = x_flat[c * TL:(c + 1) * TL]
        im["wq"] = bf(wq_s[:, :, hc])
        im["wk"] = bf(wk_s[:, :, hc])
        im["wv"] = bf(wv_s[:, :, hc])
        in_maps.append(im)
    return in_maps, blm_f


def kernel(**inputs):
    nc = _build()
    in_maps, blm_f = _prep_inputs(inputs)
    res = run_bass_kernel_spmd(nc, in_maps, list(range(NCORES)))
    full = np.zeros((B * T, V), dtype=np.float32)
    for c in range(NCORES):
        full[c * TL:(c + 1) * TL, :] = np.asarray(
            res.results[c]["logits"], dtype=np.float32
        )
    full += blm_f[None, :]
    return full.reshape(B, T, V)


# revision 35
# speedup vs baseline: 1.0535x; 1.0535x over previous
"""GPT forward pass on 8 Trainium2 NeuronCores — v2 (head-sharded attention).

Trunk (LN/FFN/residual/lm_head) is token-parallel: core c owns token tile c
(128 tokens) of each of the 4 sequences. Attention is Megatron head-sharded:
core c owns global heads {2c, 2c+1} and computes full causal attention for
those heads over all 4096 tokens — every (head, seq) unit has the identical
causal block structure (q-tile qt needs qt+1 k-tiles), so the SPMD program
is uniform across cores while skipping all fully-masked blocks (36/64).

Per layer: LN1 -> AllGather h^T (1 MB payload) -> Q/K/V for my 2 heads over
all tokens (SBUF-resident, no DRAM round-trip) -> causal attention ->
row-parallel Wo partial -> bf16 ReduceScatter back to token owners ->
residual -> LN2 -> FFN (token-parallel). lm_head is token-sharded: each
core streams the full lnf-folded Wlm and emits bf16 logits for its 512
tokens; blm is added on the host.

LN gains are folded into consuming weights host-side (exact); ln2/lnf
biases into b1/blm (exact); projection bias b2 enters the PSUM via a K=1
ones-row matmul; bo is added after the ReduceScatter. All matmuls bf16
with fp32 PSUM; residual stream and softmax stats stay fp32. Softmax is
transposed-layout with denominators from a ones-column of V (no max
subtraction needed at these scales — matches the reference to ~5e-3).
"""

import os
import sys

for _p in ("/opt/trn_rl_repo",):
    if os.path.isdir(_p) and _p not in sys.path:
        sys.path.insert(0, _p)

import numpy as np
import ml_dtypes

BF16NP = ml_dtypes.bfloat16

import concourse.bass as bass
import concourse.mybir as mybir
import concourse.tile as tile
from concourse import bacc
from concourse.bass_utils import run_bass_kernel_spmd
from concourse.masks import make_identity

F32 = mybir.dt.float32
BF = mybir.dt.bfloat16
AF = mybir.ActivationFunctionType

V, C, T, H, L, B = 32000, 1024, 1024, 16, 4, 4
HD = C // H          # 64
FF = 4 * C           # 4096
NCORES = 8
TL = 512             # local tokens per core (4 seqs x 128)
TT = B * T // 1      # 4096 total tokens (seq-major: t = s*1024 + pos)
SEQ = B
NT = TL // 128       # 4 local t-tiles; tile tt = seq tt
NCT = C // 128       # 8 c-tiles
NFT = FF // 128      # 32 f-tiles
NTB = TT // 128      # 32 global t-blocks
NCH = 64             # vocab chunks
VCW = V // NCH       # 500
LN_EPS = 1e-5

_prog_cache = {}


def _ap(t, offset, pattern):
    return bass.AP(tensor=t.tensor if isinstance(t, bass.AP) else t, offset=offset, ap=pattern)


def _build(LL=L, sim=False):
    key = (LL, sim)
    if key in _prog_cache:
        return _prog_cache[key]

    nc = bacc.Bacc("TRN2", target_bir_lowering=False, debug=False, num_devices=NCORES)

    x0 = nc.dram_tensor("x0", [TL, C], F32, kind="ExternalInput")
    mask_d = nc.dram_tensor("maskd", [128, 128], BF, kind="ExternalInput")
    wq_d = nc.dram_tensor("wq", [L, C, 128], BF, kind="ExternalInput")   # my head cols
    wk_d = nc.dram_tensor("wk", [L, C, 128], BF, kind="ExternalInput")
    wv_d = nc.dram_tensor("wv", [L, C, 128], BF, kind="ExternalInput")
    wo_d = nc.dram_tensor("wo", [L, C, C], BF, kind="ExternalInput")
    w1_d = nc.dram_tensor("w1", [L, C, FF], BF, kind="ExternalInput")
    w2_d = nc.dram_tensor("w2", [L, FF, C], BF, kind="ExternalInput")
    b1_d = nc.dram_tensor("b1", [L, FF], F32, kind="ExternalInput")
    bo_d = nc.dram_tensor("bo", [L, C], BF, kind="ExternalInput")
    b2_d = nc.dram_tensor("b2", [L, C], BF, kind="ExternalInput")
    wlm_d = nc.dram_tensor("wlm", [NCH, NCT, 128, VCW], BF, kind="ExternalInput")

    logits_d = nc.dram_tensor("logits", [TL, V], BF, kind="ExternalOutput")

    HTSZ = C * TL            # elems in one core's hT payload (1 MB bf16)

    with tile.TileContext(nc) as tc:
        import contextlib

        with contextlib.ExitStack() as ctx:
            # SBUF pools (~per-partition KB)
            const = ctx.enter_context(tc.tile_pool(name="const", bufs=1))      # .6
            xpool = ctx.enter_context(tc.tile_pool(name="x", bufs=1))          # 16
            hpool = ctx.enter_context(tc.tile_pool(name="h", bufs=5))          # 10
            tpool = ctx.enter_context(tc.tile_pool(name="hT", bufs=1))         # 8
            big = ctx.enter_context(tc.tile_pool(name="big", bufs=2))          # 64
            qkt = ctx.enter_context(tc.tile_pool(name="qkt", bufs=1))          # 16
            vsb_p = ctx.enter_context(tc.tile_pool(name="vsb", bufs=1))        # 8.3
            otm_p = ctx.enter_context(tc.tile_pool(name="otm", bufs=1))        # 8
            w4 = ctx.enter_context(tc.tile_pool(name="w4", bufs=5))            # 20
            qwp = ctx.enter_context(tc.tile_pool(name="qw", bufs=1))           # 6
            gbpool = ctx.enter_context(tc.tile_pool(name="gb", bufs=1))        # 4.5
            misc = ctx.enter_context(tc.tile_pool(name="misc", bufs=2))        # .6
            pt_pool = ctx.enter_context(tc.tile_pool(name="pt", bufs=1))       # 9
            oraw_pool = ctx.enter_context(tc.tile_pool(name="oraw", bufs=2))   # 8
            os_pool = ctx.enter_context(tc.tile_pool(name="oS", bufs=2))       # 4
            rcp_pool = ctx.enter_context(tc.tile_pool(name="rcp", bufs=1))     # 4
            rb_pool = ctx.enter_context(tc.tile_pool(name="rb", bufs=2))       # 8
            pd_pool = ctx.enter_context(tc.tile_pool(name="pd", bufs=3))       # 6
            rcv_pool = ctx.enter_context(tc.tile_pool(name="rcv", bufs=2))     # 4
            lgout = ctx.enter_context(tc.tile_pool(name="lgout", bufs=2))      # 8
            ps_acc = ctx.enter_context(tc.tile_pool(name="psacc", bufs=4, space="PSUM"))
            ps_st = ctx.enter_context(tc.tile_pool(name="psst", bufs=2, space="PSUM"))
            ps_ov = ctx.enter_context(tc.tile_pool(name="psov", bufs=2, space="PSUM"))
            dram = ctx.enter_context(tc.tile_pool(name="dram", bufs=1, space="DRAM"))

            ident = const.tile([128, 128], BF, name="ident")
            make_identity(nc, ident)
            eps_t = const.tile([128, 1], F32, name="eps")
            nc.vector.memset(eps_t[:], LN_EPS)
            mask_t = const.tile([128, 128], BF, name="mask")
            nc.sync.dma_start(out=mask_t[:], in_=mask_d[:])
            ones1 = const.tile([1, 128], BF, name="ones1")
            nc.vector.memset(ones1[:], 1.0)

            hT_loc = dram.tile([HTSZ], BF, name="hT_loc")
            o_loc = dram.tile([NCORES * 128 * 512], BF, name="o_loc")

            # persistent residual stream fp32: tile tt = seq tt
            x_t = [xpool.tile([128, C], F32, tag=f"x{tt}", name=f"x{tt}") for tt in range(NT)]
            for tt in range(NT):
                nc.sync.dma_start(out=x_t[tt][:], in_=x0[tt * 128:(tt + 1) * 128, :])

            def emit_ln():
                """x_t -> (x-m)*rstd bf16, transposed hT tiles (g/b folded away)."""
                h_tiles = []
                for tt in range(NT):
                    stats = misc.tile([128, 2, 6], F32, name="stats", tag="stats")
                    xv = x_t[tt][:].rearrange("p (s d) -> p s d", s=2)
                    nc.vector.bn_stats(out=stats[:, 0, :], in_=xv[:, 0, :])
                    nc.vector.bn_stats(out=stats[:, 1, :], in_=xv[:, 1, :])
                    mv = misc.tile([128, 2], F32, name="mv", tag="mv")
                    nc.vector.bn_aggr(out=mv[:], in_=stats[:])
                    rstd = misc.tile([128, 1], F32, name="rstd", tag="rstd")
                    nc.scalar.activation(rstd[:], mv[:, 1:2], AF.Sqrt, bias=eps_t[:])
                    nc.vector.reciprocal(rstd[:], rstd[:])
                    h = hpool.tile([128, C], BF, tag="h", name="h")
                    nc.vector.tensor_scalar(
                        out=h[:], in0=x_t[tt][:], scalar1=mv[:, 0:1], scalar2=rstd[:],
                        op0=mybir.AluOpType.subtract, op1=mybir.AluOpType.mult,
                    )
                    h_tiles.append(h)
                hT_all = tpool.tile([128, NCT, 512], BF, tag="hTall", name="hTall")
                for ct in range(NCT):
                    pst = ps_st.tile([128, 512], BF, tag="st", name="pst")
                    for tt in range(NT):
                        nc.tensor.transpose(
                            pst[:, tt * 128:(tt + 1) * 128],
                            h_tiles[tt][:, ct * 128:(ct + 1) * 128],
                            ident[:],
                        )
                    nc.vector.tensor_copy(out=hT_all[:, ct, :], in_=pst[:])
                return hT_all

            for l in range(LL):
                lw = l % L
                # per-layer bias/const tiles (issued early; Pool queue quiet)
                b2_r = gbpool.tile([1, C], BF, tag="b2r", name="b2_r")
                nc.sync.dma_start(out=b2_r[:], in_=b2_d[lw:lw + 1, :])
                bo_r = gbpool.tile([1, C], BF, tag="bor", name="bo_r")
                nc.sync.dma_start(out=bo_r[:], in_=bo_d[lw:lw + 1, :])
                b1_t = misc.tile([128, NFT], F32, tag="b1", name="b1_t")
                nc.gpsimd.dma_start(
                    out=b1_t[:], in_=_ap(b1_d, lw * FF, [[1, 128], [128, NFT]])
                )
                # my-head projection weights [128c-in-ct, ct, 128d] — one DMA each
                wq_t = qwp.tile([128, NCT, 128], BF, tag="qw", name="wq_t")
                wk_t = qwp.tile([128, NCT, 128], BF, tag="kw", name="wk_t")
                wv_t = qwp.tile([128, NCT, 128], BF, tag="vw", name="wv_t")
                for wt, wd in ((wq_t, wq_d), (wk_t, wk_d), (wv_t, wv_d)):
                    nc.sync.dma_start(
                        out=wt[:],
                        in_=_ap(wd[0:1, 0:1, 0:1], lw * C * 128,
                                [[128, 128], [128 * 128, NCT], [1, 128]]),
                    )

                # ---- LN1 -> hT, publish (one DMA), AllGather ----
                hT = emit_ln()
                nc.sync.dma_start(
                    out=_ap(hT_loc, 0, [[512, 128], [128 * 512, NCT], [1, 512]]),
                    in_=hT[:],
                )
                hT_full = dram.tile(
                    [NCORES * HTSZ], BF,
                    addr_space="Local" if sim else "Shared", name=f"hT_full{l}",
                )
                if sim:
                    nc.sync.dma_start(
                        out=_ap(hT_full, 0, [[2048, HTSZ // 2048], [1, 2048]]),
                        in_=_ap(hT_loc, 0, [[2048, HTSZ // 2048], [1, 2048]]),
                    )
                else:
                    nc.gpsimd.collective_compute(
                        "AllGather",
                        mybir.AluOpType.bypass,
                        replica_groups=[list(range(NCORES))],
                        ins=[_ap(hT_loc, 0, [[2048, HTSZ // 2048], [1, 2048]])],
                        outs=[_ap(hT_full, 0, [[2048, NCORES * HTSZ // 2048], [1, 2048]])],
                    )

                # gathered h^T lands per rank-chunk so QKV(seq s) starts after
                # 2 chunks, and attention(s) overlaps QKV(s+1)
                hT_sb = [None, None]

                def load_chunk(ch):
                    half = ch // 4
                    if hT_sb[half] is None:
                        hT_sb[half] = big.tile(
                            [128, 4, NCT, 512], BF, tag="big", name=f"hTsb{half}"
                        )
                    nc.sync.dma_start(
                        out=hT_sb[half][:, ch % 4, :, :],
                        in_=_ap(hT_full, ch * HTSZ, [[512, 128], [128 * 512, NCT], [1, 512]]),
                    )

                def htf(ct, ch):
                    return hT_sb[ch // 4][:, ch % 4, ct, :]

                qT_s, kT_s, v_ss, oTm_s = [], [], [], []
                for s in range(SEQ):
                    qT_s.append(qkt.tile([128, 1024], BF, tag=f"qT{s}", name=f"qT{s}"))
                    kT_s.append(qkt.tile([128, 1024], BF, tag=f"kT{s}", name=f"kT{s}"))
                    v_ss.append(vsb_p.tile([128, 8, 2, HD + 1], BF, tag=f"v{s}", name=f"v{s}"))
                    oTm_s.append(otm_p.tile([128, 1024], BF, tag=f"oTm{s}", name=f"oTm{s}"))

                for s in range(SEQ):
                    for hh in range(2):
                        load_chunk(2 * s + hh)
                    # K^T then Q^T for this seq (2 chunks each)
                    for dst, wt_l in ((kT_s[s], wk_t), (qT_s[s], wq_t)):
                        for hh in range(2):
                            ch = 2 * s + hh
                            ps = ps_acc.tile([128, 512], F32, tag="acc", name="acc")
                            for ct in range(NCT):
                                nc.tensor.matmul(
                                    ps[:], wt_l[:, ct, :], htf(ct, ch),
                                    start=(ct == 0), stop=(ct == NCT - 1),
                                )
                            nc.vector.tensor_copy(
                                out=dst[:, hh * 512:(hh + 1) * 512], in_=ps[:]
                            )
                    # V natural [t, my 128 d] with ones column
                    v_sb = v_ss[s]
                    nc.vector.memset(v_sb[:, :, :, HD:HD + 1], 1.0)
                    for i in range(8):
                        tb = s * 8 + i
                        ps = ps_acc.tile([128, 128], F32, tag="acc", name="psv")
                        for ct in range(NCT):
                            nc.tensor.matmul(
                                ps[:], htf(ct, tb // 4)[:, (tb % 4) * 128:(tb % 4 + 1) * 128],
                                wv_t[:, ct, :],
                                start=(ct == 0), stop=(ct == NCT - 1),
                            )
                        nc.vector.tensor_copy(out=v_sb[:, i, 0, 0:HD], in_=ps[:, 0:HD])
                        nc.vector.tensor_copy(out=v_sb[:, i, 1, 0:HD], in_=ps[:, HD:2 * HD])

                    # ---- causal attention: kt-major QK/exp (big-N, one exp
                    # unlocks all its PV uses), qt-major PV with sequential
                    # PSUM groups (HW-safe) ----
                    oraw = [
                        oraw_pool.tile([HD + 1, 1024], F32, tag="oraw", name="oraw")
                        for _ in range(2)
                    ]
                    OFF = [128 * (8 * i - i * (i - 1) // 2) for i in range(8)]

                    for hp in range(2):
                        poff = hp * HD
                        pTa = pt_pool.tile([128, 4608], BF, tag="pt", name="pt")

                        def emit_pv(qt):
                            ov = ps_ov.tile([128, 128], F32, tag="ov", name="ov")
                            for k2 in range(qt + 1):
                                nc.tensor.matmul(
                                    ov[0:HD + 1, :], v_sb[:, k2, hp, :],
                                    pTa[:, OFF[k2] + (qt - k2) * 128:OFF[k2] + (qt - k2 + 1) * 128],
                                    start=(k2 == 0), stop=(k2 == qt),
                                )
                            nc.vector.tensor_copy(
                                out=oraw[hp][:, qt * 128:(qt + 1) * 128], in_=ov[0:HD + 1, :]
                            )

                        for kt in range(8):
                            nq = 8 - kt
                            off = 0
                            while off < nq * 128:
                                cols = min(512, nq * 128 - off)
                                st = ps_st.tile([128, 512], F32, tag="st", name="st")
                                nc.tensor.matmul(
                                    st[:, 0:cols],
                                    kT_s[s][poff:poff + HD, kt * 128:(kt + 1) * 128],
                                    qT_s[s][poff:poff + HD, kt * 128 + off:kt * 128 + off + cols],
                                    start=True, stop=True,
                                )
                                nc.scalar.activation(
                                    pTa[:, OFF[kt] + off:OFF[kt] + off + cols], st[:, 0:cols], AF.Exp
                                )
                                off += cols
                            nc.vector.tensor_mul(
                                out=pTa[:, OFF[kt]:OFF[kt] + 128],
                                in0=pTa[:, OFF[kt]:OFF[kt] + 128], in1=mask_t[:],
                            )
                            if kt >= 1:
                                emit_pv(kt - 1)
                        emit_pv(6)
                        emit_pv(7)
                    for hp in range(2):
                        recips = rcp_pool.tile([1, 1024], F32, tag="recips", name="recips")
                        nc.vector.reciprocal(recips[:], oraw[hp][HD:HD + 1, :])
                        rc_b = dram.tile([1024], F32, name=f"rcb{l}_{s}_{hp}")
                        nc.sync.dma_start(out=rc_b[:], in_=recips[:])
                        rb = rb_pool.tile([HD, 1024], F32, tag="rb", name="rb")
                        nc.gpsimd.dma_start(out=rb[:], in_=_ap(rc_b, 0, [[0, HD], [1, 1024]]))
                        if hp == 0:
                            nc.gpsimd.tensor_mul(
                                out=oTm_s[s][0:HD, :], in0=oraw[hp][0:HD, :], in1=rb[:]
                            )
                        else:
                            oS = os_pool.tile([HD, 1024], BF, tag="oS", name="oS")
                            nc.gpsimd.tensor_mul(out=oS[:], in0=oraw[hp][0:HD, :], in1=rb[:])
                            nc.sync.dma_start(out=oTm_s[s][HD:128, :], in_=oS[:])
                    nc.sync.dma_start(
                        out=_ap(o_loc, 2 * s * 128 * 512, [[512, 128], [128 * 512, 2], [1, 512]]),
                        in_=oTm_s[s][:],
                    )
                o_recv = dram.tile([NCORES * 128 * 512], BF, name=f"orecv{l}")
                if sim:
                    nc.sync.dma_start(
                        out=_ap(o_recv, 0, [[2048, NCORES * 128 * 512 // 2048], [1, 2048]]),
                        in_=_ap(o_loc, 0, [[2048, NCORES * 128 * 512 // 2048], [1, 2048]]),
                    )
                else:
                    nc.gpsimd.collective_compute(
                        "AllToAll",
                        mybir.AluOpType.bypass,
                        replica_groups=[list(range(NCORES))],
                        ins=[_ap(o_loc, 0, [[2048, NCORES * 128 * 512 // 2048], [1, 2048]])],
                        outs=[_ap(o_recv, 0, [[2048, NCORES * 128 * 512 // 2048], [1, 2048]])],
                    )
                orv = [otm_p.tile([128, 4, 512], BF, tag=f"orv{h_}", name="orv")
                       for h_ in range(2)]
                for h_ in range(2):
                    nc.sync.dma_start(
                        out=orv[h_][:],
                        in_=_ap(o_recv, h_ * 4 * 128 * 512,
                                [[512, 128], [128 * 512, 4], [1, 512]]),
                    )
                for nf in range(2):
                    pss = [ps_acc.tile([128, 512], F32, tag="acc", name="acc") for _ in range(4)]
                    for cg in range(2):
                        wt = w4.tile([128, 4, 512], BF, tag="w", name="wot")
                        nc.sync.dma_start(
                            out=wt[:],
                            in_=_ap(wo_d[0:1, 0:1, 0:1],
                                    lw * C * C + cg * 512 * C + nf * 512,
                                    [[C, 128], [128 * C, 4], [1, 512]]),
                        )
                        for ci in range(4):
                            ct = cg * 4 + ci
                            for tt in range(NT):
                                nc.tensor.matmul(
                                    pss[tt][:],
                                    orv[cg][:, ci, tt * 128:(tt + 1) * 128], wt[:, ci, :],
                                    start=(ct == 0), stop=False,
                                )
                    for tt in range(NT):
                        nc.tensor.matmul(
                            pss[tt][:], ones1[:], bo_r[:, nf * 512:(nf + 1) * 512],
                            start=False, stop=True,
                        )
                        xs = x_t[tt][:, nf * 512:(nf + 1) * 512]
                        nc.vector.tensor_add(out=xs, in0=xs, in1=pss[tt][:])

                # ---- FFN (token-parallel, biases in-psum / in-activation) ----
                h2T = emit_ln()
                ug = big.tile([128, NFT, 512], BF, tag="big", name="ug")
                for fg in range(8):
                    pss = [ps_acc.tile([128, 512], F32, tag="acc", name="acc") for _ in range(4)]
                    for hf in range(2):
                        wt = w4.tile([128, 4, 512], BF, tag="w", name="w1t")
                        nc.sync.dma_start(
                            out=wt[:],
                            in_=_ap(w1_d[0:1, 0:1, 0:1],
                                    lw * C * FF + hf * 512 * FF + fg * 512,
                                    [[FF, 128], [128 * FF, 4], [1, 512]]),
                        )
                        for ci in range(4):
                            ct = hf * 4 + ci
                            for f4 in range(4):
                                nc.tensor.matmul(
                                    pss[f4][:], wt[:, ci, f4 * 128:(f4 + 1) * 128], h2T[:, ct, :],
                                    start=(ct == 0), stop=(ct == NCT - 1),
                                )
                    for f4 in range(4):
                        ft = fg * 4 + f4
                        nc.scalar.activation(
                            ug[:, ft, :], pss[f4][:], AF.Gelu, bias=b1_t[:, ft:ft + 1]
                        )
                for nf in range(2):
                    pss = [ps_acc.tile([128, 512], F32, tag="acc", name="acc") for _ in range(4)]
                    for g8 in range(8):
                        wt = w4.tile([128, 4, 512], BF, tag="w", name="w2t")
                        nc.sync.dma_start(
                            out=wt[:],
                            in_=_ap(w2_d[0:1, 0:1, 0:1],
                                    lw * FF * C + g8 * 512 * C + nf * 512,
                                    [[C, 128], [128 * C, 4], [1, 512]]),
                        )
                        for fi in range(4):
                            ft = g8 * 4 + fi
                            for tt in range(NT):
                                nc.tensor.matmul(
                                    pss[tt][:], ug[:, ft, tt * 128:(tt + 1) * 128], wt[:, fi, :],
                                    start=(ft == 0), stop=False,
                                )
                    for tt in range(NT):
                        nc.tensor.matmul(
                            pss[tt][:], ones1[:], b2_r[:, nf * 512:(nf + 1) * 512],
                            start=False, stop=True,
                        )
                        xs = x_t[tt][:, nf * 512:(nf + 1) * 512]
                        nc.vector.tensor_add(out=xs, in0=xs, in1=pss[tt][:])

            # ---- final LN (folded) + token-sharded lm_head ----
            hfT = emit_ln()
            for ch in range(NCH):
                wlm_c = big.tile([128, NCT, VCW], BF, tag="big", name="wlm_c")
                nc.sync.dma_start(
                    out=wlm_c[:],
                    in_=_ap(wlm_d[0:1, 0:1, 0:1, 0:1], ch * NCT * 128 * VCW,
                            [[VCW, 128], [128 * VCW, NCT], [1, VCW]]),
                )
                for th in range(2):
                    lg = lgout.tile([128, 2, VCW], BF, tag="lg", name="lg")
                    for ti in range(2):
                        tt = th * 2 + ti
                        ps = ps_acc.tile([128, VCW], F32, tag="acc", name="acc")
                        for ct in range(NCT):
                            nc.tensor.matmul(
                                ps[:], hfT[:, ct, tt * 128:(tt + 1) * 128], wlm_c[:, ct, :],
                                start=(ct == 0), stop=(ct == NCT - 1),
                            )
                        if ti % 2 == 0:
                            nc.vector.tensor_copy(out=lg[:, ti, :], in_=ps[:])
                        else:
                            nc.scalar.activation(lg[:, ti, :], ps[:], AF.Copy)
                    nc.sync.dma_start(
                        out=_ap(logits_d[0:1, 0:1], th * 2 * 128 * V + ch * VCW,
                                [[V, 128], [128 * V, 2], [1, VCW]]),
                        in_=lg[:],
                    )

    nc.compile()
    _prog_cache[key] = nc
    return nc


def _prep_inputs(inputs):
    f = {k: np.asarray(v) for k, v in inputs.items()}
    idx = f["idx"].astype(np.int64)
    emb = f["emb"].astype(np.float32)
    pos = f["pos_enc"].astype(np.float32)
    x_full = emb[idx] + pos[None, :, :]          # [B,T,C] f32

    scale = HD ** -0.5
    g1 = f["ln1_g"].astype(np.float32)
    b1ln = f["ln1_b"].astype(np.float32)
    g2 = f["ln2_g"].astype(np.float32)
    b2ln = f["ln2_b"].astype(np.float32)
    gf = f["lnf_g"].astype(np.float32)
    bfln = f["lnf_b"].astype(np.float32)
    W1 = f["W1"].astype(np.float32)
    Wv = f["Wv"].astype(np.float32)
    Wo = f["Wo"].astype(np.float32)
    Wlm = f["Wlm"].astype(np.float32)

    bf = lambda a: np.ascontiguousarray(a, dtype=np.float32).astype(BF16NP)
    b1_f = f["b1"].astype(np.float32) + np.einsum("lc,lcf->lf", b2ln, W1)
    bo_f = f["bo"].astype(np.float32) + np.einsum(
        "ld,ldc->lc", np.einsum("lc,lcd->ld", b1ln, Wv), Wo
    )
    blm_f = f["blm"].astype(np.float32) + bfln @ Wlm

    wq_s = f["Wq"].astype(np.float32) * scale * g1[:, :, None]
    wk_s = f["Wk"].astype(np.float32) * g1[:, :, None]
    wv_s = Wv * g1[:, :, None]
    wlm_blocks = np.ascontiguousarray(
        (Wlm * gf[:, None]).reshape(NCT, 128, NCH, VCW).transpose(2, 0, 1, 3)
    ).astype(BF16NP)

    shared = {
        "w1": bf(W1 * g2[:, :, None]),
        "w2": bf(f["W2"]),
        "b1": b1_f.astype(np.float32),
        "bo": bf(bo_f),
        "wo": bf(Wo),
        "b2": bf(f["b2"]),
        "wlm": wlm_blocks,
        "maskd": np.triu(np.ones((128, 128), dtype=np.float32)).astype(BF16NP),
    }

    x_flat = np.ascontiguousarray(x_full.reshape(B * T, C), dtype=np.float32)
    in_maps = []
    for c in range(NCORES):
        hc = slice(c * 128, (c + 1) * 128)
        im = dict(shared)
        # core c owns flat tokens [c*512, (c+1)*512) — seq c//2, half c%2 —
        # so gathered-hT chunk r is exactly rank r's contiguous token block
        im["x0"] = x_flat[c * TL:(c + 1) * TL]
        im["wq"] = bf(wq_s[:, :, hc])
        im["wk"] = bf(wk_s[:, :, hc])
        im["wv"] = bf(wv_s[:, :, hc])
        in_maps.append(im)
    return in_maps, blm_f


def kernel(**inputs):
    nc = _build()
    in_maps, blm_f = _prep_inputs(inputs)
    res = run_bass_kernel_spmd(nc, in_maps, list(range(NCORES)))
    full = np.zeros((B * T, V), dtype=np.float32)
    for c in range(NCORES):
        full[c * TL:(c + 1) * TL, :] = np.asarray(
            res.results[c]["logits"], dtype=np.float32
        )
    full += blm_f[None, :]
    return full.reshape(B, T, V)


# revision 36
# speedup vs baseline: 1.0825x; 1.0276x over previous
"""GPT forward pass on 8 Trainium2 NeuronCores — v2 (head-sharded attention).

Trunk (LN/FFN/residual/lm_head) is token-parallel: core c owns token tile c
(128 tokens) of each of the 4 sequences. Attention is Megatron head-sharded:
core c owns global heads {2c, 2c+1} and computes full causal attention for
those heads over all 4096 tokens — every (head, seq) unit has the identical
causal block structure (q-tile qt needs qt+1 k-tiles), so the SPMD program
is uniform across cores while skipping all fully-masked blocks (36/64).

Per layer: LN1 -> AllGather h^T (1 MB payload) -> Q/K/V for my 2 heads over
all tokens (SBUF-resident, no DRAM round-trip) -> causal attention ->
row-parallel Wo partial -> bf16 ReduceScatter back to token owners ->
residual -> LN2 -> FFN (token-parallel). lm_head is token-sharded: each
core streams the full lnf-folded Wlm and emits bf16 logits for its 512
tokens; blm is added on the host.

LN gains are folded into consuming weights host-side (exact); ln2/lnf
biases into b1/blm (exact); projection bias b2 enters the PSUM via a K=1
ones-row matmul; bo is added after the ReduceScatter. All matmuls bf16
with fp32 PSUM; residual stream and softmax stats stay fp32. Softmax is
transposed-layout with denominators from a ones-column of V (no max
subtraction needed at these scales — matches the reference to ~5e-3).
"""

import os
import sys

for _p in ("/opt/trn_rl_repo",):
    if os.path.isdir(_p) and _p not in sys.path:
        sys.path.insert(0, _p)

import numpy as np
import ml_dtypes

BF16NP = ml_dtypes.bfloat16

import concourse.bass as bass
import concourse.mybir as mybir
import concourse.tile as tile
from concourse import bacc
from concourse.bass_utils import run_bass_kernel_spmd
from concourse.masks import make_identity

F32 = mybir.dt.float32
BF = mybir.dt.bfloat16
AF = mybir.ActivationFunctionType

V, C, T, H, L, B = 32000, 1024, 1024, 16, 4, 4
HD = C // H          # 64
FF = 4 * C           # 4096
NCORES = 8
TL = 512             # local tokens per core (4 seqs x 128)
TT = B * T // 1      # 4096 total tokens (seq-major: t = s*1024 + pos)
SEQ = B
NT = TL // 128       # 4 local t-tiles; tile tt = seq tt
NCT = C // 128       # 8 c-tiles
NFT = FF // 128      # 32 f-tiles
NTB = TT // 128      # 32 global t-blocks
NCH = 64             # vocab chunks
VCW = V // NCH       # 500
LN_EPS = 1e-5

_prog_cache = {}


def _ap(t, offset, pattern):
    return bass.AP(tensor=t.tensor if isinstance(t, bass.AP) else t, offset=offset, ap=pattern)


def _build(LL=L, sim=False):
    key = (LL, sim)
    if key in _prog_cache:
        return _prog_cache[key]

    nc = bacc.Bacc("TRN2", target_bir_lowering=False, debug=False, num_devices=NCORES)

    x0 = nc.dram_tensor("x0", [TL, C], F32, kind="ExternalInput")
    mask_d = nc.dram_tensor("maskd", [128, 128], BF, kind="ExternalInput")
    wq_d = nc.dram_tensor("wq", [L, C, 128], BF, kind="ExternalInput")   # my head cols
    wk_d = nc.dram_tensor("wk", [L, C, 128], BF, kind="ExternalInput")
    wv_d = nc.dram_tensor("wv", [L, C, 128], BF, kind="ExternalInput")
    wo_d = nc.dram_tensor("wo", [L, C, C], BF, kind="ExternalInput")
    w1_d = nc.dram_tensor("w1", [L, C, FF], BF, kind="ExternalInput")
    w2_d = nc.dram_tensor("w2", [L, FF, C], BF, kind="ExternalInput")
    b1_d = nc.dram_tensor("b1", [L, FF], F32, kind="ExternalInput")
    bo_d = nc.dram_tensor("bo", [L, C], BF, kind="ExternalInput")
    b2_d = nc.dram_tensor("b2", [L, C], BF, kind="ExternalInput")
    wlm_d = nc.dram_tensor("wlm", [NCH, NCT, 128, VCW], BF, kind="ExternalInput")

    logits_d = nc.dram_tensor("logits", [TL, V], BF, kind="ExternalOutput")

    HTSZ = C * TL            # elems in one core's hT payload (1 MB bf16)

    with tile.TileContext(nc) as tc:
        import contextlib

        with contextlib.ExitStack() as ctx:
            # SBUF pools (~per-partition KB)
            const = ctx.enter_context(tc.tile_pool(name="const", bufs=1))      # .6
            xpool = ctx.enter_context(tc.tile_pool(name="x", bufs=1))          # 16
            hpool = ctx.enter_context(tc.tile_pool(name="h", bufs=5))          # 10
            tpool = ctx.enter_context(tc.tile_pool(name="hT", bufs=1))         # 8
            big = ctx.enter_context(tc.tile_pool(name="big", bufs=2))          # 64
            qkt = ctx.enter_context(tc.tile_pool(name="qkt", bufs=1))          # 16
            vsb_p = ctx.enter_context(tc.tile_pool(name="vsb", bufs=1))        # 8.3
            otm_p = ctx.enter_context(tc.tile_pool(name="otm", bufs=1))        # 8
            w4 = ctx.enter_context(tc.tile_pool(name="w4", bufs=5))            # 20
            qwp = ctx.enter_context(tc.tile_pool(name="qw", bufs=1))           # 6
            gbpool = ctx.enter_context(tc.tile_pool(name="gb", bufs=1))        # 4.5
            misc = ctx.enter_context(tc.tile_pool(name="misc", bufs=2))        # .6
            pt_pool = ctx.enter_context(tc.tile_pool(name="pt", bufs=1))       # 9
            oraw_pool = ctx.enter_context(tc.tile_pool(name="oraw", bufs=2))   # 8
            os_pool = ctx.enter_context(tc.tile_pool(name="oS", bufs=2))       # 4
            rcp_pool = ctx.enter_context(tc.tile_pool(name="rcp", bufs=1))     # 4
            rb_pool = ctx.enter_context(tc.tile_pool(name="rb", bufs=2))       # 8
            pd_pool = ctx.enter_context(tc.tile_pool(name="pd", bufs=3))       # 6
            rcv_pool = ctx.enter_context(tc.tile_pool(name="rcv", bufs=2))     # 4
            lgout = ctx.enter_context(tc.tile_pool(name="lgout", bufs=2))      # 8
            ps_acc = ctx.enter_context(tc.tile_pool(name="psacc", bufs=4, space="PSUM"))
            ps_st = ctx.enter_context(tc.tile_pool(name="psst", bufs=2, space="PSUM"))
            ps_ov = ctx.enter_context(tc.tile_pool(name="psov", bufs=2, space="PSUM"))
            dram = ctx.enter_context(tc.tile_pool(name="dram", bufs=1, space="DRAM"))

            ident = const.tile([128, 128], BF, name="ident")
            make_identity(nc, ident)
            eps_t = const.tile([128, 1], F32, name="eps")
            nc.vector.memset(eps_t[:], LN_EPS)
            mask_t = const.tile([128, 128], BF, name="mask")
            nc.sync.dma_start(out=mask_t[:], in_=mask_d[:])
            ones1 = const.tile([1, 128], BF, name="ones1")
            nc.vector.memset(ones1[:], 1.0)

            hT_loc = dram.tile([HTSZ], BF, name="hT_loc")
            o_loc = dram.tile([NCORES * 128 * 512], BF, name="o_loc")

            # persistent residual stream fp32: tile tt = seq tt
            x_t = [xpool.tile([128, C], F32, tag=f"x{tt}", name=f"x{tt}") for tt in range(NT)]
            for tt in range(NT):
                nc.sync.dma_start(out=x_t[tt][:], in_=x0[tt * 128:(tt + 1) * 128, :])

            def emit_ln():
                """x_t -> (x-m)*rstd bf16, transposed hT tiles (g/b folded away)."""
                h_tiles = []
                for tt in range(NT):
                    stats = misc.tile([128, 2, 6], F32, name="stats", tag="stats")
                    xv = x_t[tt][:].rearrange("p (s d) -> p s d", s=2)
                    nc.vector.bn_stats(out=stats[:, 0, :], in_=xv[:, 0, :])
                    nc.vector.bn_stats(out=stats[:, 1, :], in_=xv[:, 1, :])
                    mv = misc.tile([128, 2], F32, name="mv", tag="mv")
                    nc.vector.bn_aggr(out=mv[:], in_=stats[:])
                    rstd = misc.tile([128, 1], F32, name="rstd", tag="rstd")
                    nc.scalar.activation(rstd[:], mv[:, 1:2], AF.Sqrt, bias=eps_t[:])
                    nc.vector.reciprocal(rstd[:], rstd[:])
                    h = hpool.tile([128, C], BF, tag="h", name="h")
                    nc.vector.tensor_scalar(
                        out=h[:], in0=x_t[tt][:], scalar1=mv[:, 0:1], scalar2=rstd[:],
                        op0=mybir.AluOpType.subtract, op1=mybir.AluOpType.mult,
                    )
                    h_tiles.append(h)
                hT_all = tpool.tile([128, NCT, 512], BF, tag="hTall", name="hTall")
                for ct in range(NCT):
                    pst = ps_st.tile([128, 512], BF, tag="st", name="pst")
                    for tt in range(NT):
                        nc.tensor.transpose(
                            pst[:, tt * 128:(tt + 1) * 128],
                            h_tiles[tt][:, ct * 128:(ct + 1) * 128],
                            ident[:],
                        )
                    nc.vector.tensor_copy(out=hT_all[:, ct, :], in_=pst[:])
                return hT_all

            for l in range(LL):
                lw = l % L
                # per-layer bias/const tiles (issued early; Pool queue quiet)
                b2_r = gbpool.tile([1, C], BF, tag="b2r", name="b2_r")
                nc.sync.dma_start(out=b2_r[:], in_=b2_d[lw:lw + 1, :])
                bo_r = gbpool.tile([1, C], BF, tag="bor", name="bo_r")
                nc.sync.dma_start(out=bo_r[:], in_=bo_d[lw:lw + 1, :])
                b1_t = misc.tile([128, NFT], F32, tag="b1", name="b1_t")
                nc.gpsimd.dma_start(
                    out=b1_t[:], in_=_ap(b1_d, lw * FF, [[1, 128], [128, NFT]])
                )
                # my-head projection weights [128c-in-ct, ct, 128d] — one DMA each
                wq_t = qwp.tile([128, NCT, 128], BF, tag="qw", name="wq_t")
                wk_t = qwp.tile([128, NCT, 128], BF, tag="kw", name="wk_t")
                wv_t = qwp.tile([128, NCT, 128], BF, tag="vw", name="wv_t")
                for wt, wd in ((wq_t, wq_d), (wk_t, wk_d), (wv_t, wv_d)):
                    nc.sync.dma_start(
                        out=wt[:],
                        in_=_ap(wd[0:1, 0:1, 0:1], lw * C * 128,
                                [[128, 128], [128 * 128, NCT], [1, 128]]),
                    )

                # ---- LN1 -> hT, publish (one DMA), AllGather ----
                hT = emit_ln()
                nc.sync.dma_start(
                    out=_ap(hT_loc, 0, [[512, 128], [128 * 512, NCT], [1, 512]]),
                    in_=hT[:],
                )
                hT_full = dram.tile(
                    [NCORES * HTSZ], BF,
                    addr_space="Local" if sim else "Shared", name=f"hT_full{l}",
                )
                if sim:
                    nc.sync.dma_start(
                        out=_ap(hT_full, 0, [[2048, HTSZ // 2048], [1, 2048]]),
                        in_=_ap(hT_loc, 0, [[2048, HTSZ // 2048], [1, 2048]]),
                    )
                else:
                    nc.gpsimd.collective_compute(
                        "AllGather",
                        mybir.AluOpType.bypass,
                        replica_groups=[list(range(NCORES))],
                        ins=[_ap(hT_loc, 0, [[2048, HTSZ // 2048], [1, 2048]])],
                        outs=[_ap(hT_full, 0, [[2048, NCORES * HTSZ // 2048], [1, 2048]])],
                    )

                # gathered h^T lands per rank-chunk so QKV(seq s) starts after
                # 2 chunks, and attention(s) overlaps QKV(s+1)
                hT_sb = [None, None]

                def load_chunk(ch):
                    half = ch // 4
                    if hT_sb[half] is None:
                        hT_sb[half] = big.tile(
                            [128, 4, NCT, 512], BF, tag="big", name=f"hTsb{half}"
                        )
                    nc.sync.dma_start(
                        out=hT_sb[half][:, ch % 4, :, :],
                        in_=_ap(hT_full, ch * HTSZ, [[512, 128], [128 * 512, NCT], [1, 512]]),
                    )

                def htf(ct, ch):
                    return hT_sb[ch // 4][:, ch % 4, ct, :]

                qT_s, kT_s, v_ss, oTm_s = [], [], [], []
                for s in range(SEQ):
                    qT_s.append(qkt.tile([128, 1024], BF, tag=f"qT{s}", name=f"qT{s}"))
                    kT_s.append(qkt.tile([128, 1024], BF, tag=f"kT{s}", name=f"kT{s}"))
                    v_ss.append(vsb_p.tile([128, 8, 2, HD + 1], BF, tag=f"v{s}", name=f"v{s}"))
                    oTm_s.append(otm_p.tile([128, 1024], BF, tag=f"oTm{s}", name=f"oTm{s}"))

                for s in range(SEQ):
                    for hh in range(2):
                        load_chunk(2 * s + hh)
                    # K^T then Q^T for this seq (2 chunks each)
                    for dst, wt_l in ((kT_s[s], wk_t), (qT_s[s], wq_t)):
                        for hh in range(2):
                            ch = 2 * s + hh
                            ps = ps_acc.tile([128, 512], F32, tag="acc", name="acc")
                            for ct in range(NCT):
                                nc.tensor.matmul(
                                    ps[:], wt_l[:, ct, :], htf(ct, ch),
                                    start=(ct == 0), stop=(ct == NCT - 1),
                                )
                            nc.vector.tensor_copy(
                                out=dst[:, hh * 512:(hh + 1) * 512], in_=ps[:]
                            )
                    # V natural [t, my 128 d] with ones column
                    v_sb = v_ss[s]
                    nc.vector.memset(v_sb[:, :, :, HD:HD + 1], 1.0)
                    for i in range(8):
                        tb = s * 8 + i
                        ps = ps_acc.tile([128, 128], F32, tag="acc", name="psv")
                        for ct in range(NCT):
                            nc.tensor.matmul(
                                ps[:], htf(ct, tb // 4)[:, (tb % 4) * 128:(tb % 4 + 1) * 128],
                                wv_t[:, ct, :],
                                start=(ct == 0), stop=(ct == NCT - 1),
                            )
                        nc.vector.tensor_copy(out=v_sb[:, i, 0, 0:HD], in_=ps[:, 0:HD])
                        nc.vector.tensor_copy(out=v_sb[:, i, 1, 0:HD], in_=ps[:, HD:2 * HD])

                    # ---- causal attention: kt-major QK/exp (big-N, one exp
                    # unlocks all its PV uses), qt-major PV with sequential
                    # PSUM groups (HW-safe) ----
                    oraw = [
                        oraw_pool.tile([HD + 1, 1024], F32, tag="oraw", name="oraw")
                        for _ in range(2)
                    ]
                    OFF = [128 * (8 * i - i * (i - 1) // 2) for i in range(8)]

                    for hp in range(2):
                        poff = hp * HD
                        pTa = pt_pool.tile([128, 4608], BF, tag="pt", name="pt")

                        def emit_pv(qt):
                            ov = ps_ov.tile([128, 128], F32, tag="ov", name="ov")
                            for k2 in range(qt + 1):
                                nc.tensor.matmul(
                                    ov[0:HD + 1, :], v_sb[:, k2, hp, :],
                                    pTa[:, OFF[k2] + (qt - k2) * 128:OFF[k2] + (qt - k2 + 1) * 128],
                                    start=(k2 == 0), stop=(k2 == qt),
                                )
                            nc.vector.tensor_copy(
                                out=oraw[hp][:, qt * 128:(qt + 1) * 128], in_=ov[0:HD + 1, :]
                            )

                        for kt in range(8):
                            nq = 8 - kt
                            off = 0
                            while off < nq * 128:
                                cols = min(512, nq * 128 - off)
                                st = ps_st.tile([128, 512], F32, tag="st", name="st")
                                nc.tensor.matmul(
                                    st[:, 0:cols],
                                    kT_s[s][poff:poff + HD, kt * 128:(kt + 1) * 128],
                                    qT_s[s][poff:poff + HD, kt * 128 + off:kt * 128 + off + cols],
                                    start=True, stop=True,
                                )
                                nc.scalar.activation(
                                    pTa[:, OFF[kt] + off:OFF[kt] + off + cols], st[:, 0:cols], AF.Exp
                                )
                                off += cols
                            nc.vector.tensor_mul(
                                out=pTa[:, OFF[kt]:OFF[kt] + 128],
                                in0=pTa[:, OFF[kt]:OFF[kt] + 128], in1=mask_t[:],
                            )
                            if kt >= 1:
                                emit_pv(kt - 1)
                        emit_pv(6)
                        emit_pv(7)
                    for hp in range(2):
                        recips = rcp_pool.tile([1, 1024], F32, tag="recips", name="recips")
                        nc.vector.reciprocal(recips[:], oraw[hp][HD:HD + 1, :])
                        rc_b = dram.tile([1024], F32, name=f"rcb{l}_{s}_{hp}")
                        nc.sync.dma_start(out=rc_b[:], in_=recips[:])
                        rb = rb_pool.tile([HD, 1024], F32, tag="rb", name="rb")
                        nc.gpsimd.dma_start(out=rb[:], in_=_ap(rc_b, 0, [[0, HD], [1, 1024]]))
                        if hp == 0:
                            nc.gpsimd.tensor_mul(
                                out=oTm_s[s][0:HD, :], in0=oraw[hp][0:HD, :], in1=rb[:]
                            )
                        else:
                            oS = os_pool.tile([HD, 1024], BF, tag="oS", name="oS")
                            nc.gpsimd.tensor_mul(out=oS[:], in0=oraw[hp][0:HD, :], in1=rb[:])
                            nc.sync.dma_start(out=oTm_s[s][HD:128, :], in_=oS[:])
                    nc.sync.dma_start(
                        out=_ap(o_loc, 2 * s * 128 * 512, [[512, 128], [128 * 512, 2], [1, 512]]),
                        in_=oTm_s[s][:],
                    )
                o_recv = dram.tile([NCORES * 128 * 512], BF, name=f"orecv{l}")
                if sim:
                    nc.sync.dma_start(
                        out=_ap(o_recv, 0, [[2048, NCORES * 128 * 512 // 2048], [1, 2048]]),
                        in_=_ap(o_loc, 0, [[2048, NCORES * 128 * 512 // 2048], [1, 2048]]),
                    )
                else:
                    nc.gpsimd.collective_compute(
                        "AllToAll",
                        mybir.AluOpType.bypass,
                        replica_groups=[list(range(NCORES))],
                        ins=[_ap(o_loc, 0, [[2048, NCORES * 128 * 512 // 2048], [1, 2048]])],
                        outs=[_ap(o_recv, 0, [[2048, NCORES * 128 * 512 // 2048], [1, 2048]])],
                    )
                orv = [otm_p.tile([128, 4, 512], BF, tag=f"orv{h_}", name="orv")
                       for h_ in range(2)]
                for h_ in range(2):
                    nc.sync.dma_start(
                        out=orv[h_][:],
                        in_=_ap(o_recv, h_ * 4 * 128 * 512,
                                [[512, 128], [128 * 512, 4], [1, 512]]),
                    )
                wo_ts = []
                for nf in range(2):
                    for cg in range(2):
                        wt = w4.tile([128, 4, 512], BF, tag="w", name="wot")
                        nc.sync.dma_start(
                            out=wt[:],
                            in_=_ap(wo_d[0:1, 0:1, 0:1],
                                    lw * C * C + cg * 512 * C + nf * 512,
                                    [[C, 128], [128 * C, 4], [1, 512]]),
                        )
                        wo_ts.append(wt)
                # tt-outer Wo + residual, LN2 stats fused per tile
                h2_tiles = []
                for tt in range(NT):
                    for nf in range(2):
                        ps = ps_acc.tile([128, 512], F32, tag="acc", name="acc")
                        for cg in range(2):
                            wt = wo_ts[nf * 2 + cg]
                            for ci in range(4):
                                nc.tensor.matmul(
                                    ps[:], orv[cg][:, ci, tt * 128:(tt + 1) * 128], wt[:, ci, :],
                                    start=(cg == 0 and ci == 0), stop=False,
                                )
                        nc.tensor.matmul(
                            ps[:], ones1[:], bo_r[:, nf * 512:(nf + 1) * 512],
                            start=False, stop=True,
                        )
                        xs = x_t[tt][:, nf * 512:(nf + 1) * 512]
                        nc.vector.tensor_add(out=xs, in0=xs, in1=ps[:])
                    stats = misc.tile([128, 2, 6], F32, name="stats", tag="stats")
                    xv = x_t[tt][:].rearrange("p (s d) -> p s d", s=2)
                    nc.vector.bn_stats(out=stats[:, 0, :], in_=xv[:, 0, :])
                    nc.vector.bn_stats(out=stats[:, 1, :], in_=xv[:, 1, :])
                    mv = misc.tile([128, 2], F32, name="mv", tag="mv")
                    nc.vector.bn_aggr(out=mv[:], in_=stats[:])
                    rstd = misc.tile([128, 1], F32, name="rstd", tag="rstd")
                    nc.scalar.activation(rstd[:], mv[:, 1:2], AF.Sqrt, bias=eps_t[:])
                    nc.vector.reciprocal(rstd[:], rstd[:])
                    h = hpool.tile([128, C], BF, tag="h", name="h")
                    nc.vector.tensor_scalar(
                        out=h[:], in0=x_t[tt][:], scalar1=mv[:, 0:1], scalar2=rstd[:],
                        op0=mybir.AluOpType.subtract, op1=mybir.AluOpType.mult,
                    )
                    h2_tiles.append(h)
                h2T = tpool.tile([128, NCT, 512], BF, tag="hTall", name="h2Tall")
                for ct in range(NCT):
                    pst = ps_st.tile([128, 512], BF, tag="st", name="pst")
                    for tt in range(NT):
                        nc.tensor.transpose(
                            pst[:, tt * 128:(tt + 1) * 128],
                            h2_tiles[tt][:, ct * 128:(ct + 1) * 128],
                            ident[:],
                        )
                    nc.vector.tensor_copy(out=h2T[:, ct, :], in_=pst[:])

                # ---- FFN (token-parallel, biases in-psum / in-activation) ----
                ug = big.tile([128, NFT, 512], BF, tag="big", name="ug")
                for fg in range(8):
                    pss = [ps_acc.tile([128, 512], F32, tag="acc", name="acc") for _ in range(4)]
                    for hf in range(2):
                        wt = w4.tile([128, 4, 512], BF, tag="w", name="w1t")
                        nc.sync.dma_start(
                            out=wt[:],
                            in_=_ap(w1_d[0:1, 0:1, 0:1],
                                    lw * C * FF + hf * 512 * FF + fg * 512,
                                    [[FF, 128], [128 * FF, 4], [1, 512]]),
                        )
                        for ci in range(4):
                            ct = hf * 4 + ci
                            for f4 in range(4):
                                nc.tensor.matmul(
                                    pss[f4][:], wt[:, ci, f4 * 128:(f4 + 1) * 128], h2T[:, ct, :],
                                    start=(ct == 0), stop=(ct == NCT - 1),
                                )
                    for f4 in range(4):
                        ft = fg * 4 + f4
                        nc.scalar.activation(
                            ug[:, ft, :], pss[f4][:], AF.Gelu, bias=b1_t[:, ft:ft + 1]
                        )
                for nf in range(2):
                    pss = [ps_acc.tile([128, 512], F32, tag="acc", name="acc") for _ in range(4)]
                    for g8 in range(8):
                        wt = w4.tile([128, 4, 512], BF, tag="w", name="w2t")
                        nc.sync.dma_start(
                            out=wt[:],
                            in_=_ap(w2_d[0:1, 0:1, 0:1],
                                    lw * FF * C + g8 * 512 * C + nf * 512,
                                    [[C, 128], [128 * C, 4], [1, 512]]),
                        )
                        for fi in range(4):
                            ft = g8 * 4 + fi
                            for tt in range(NT):
                                nc.tensor.matmul(
                                    pss[tt][:], ug[:, ft, tt * 128:(tt + 1) * 128], wt[:, fi, :],
                                    start=(ft == 0), stop=False,
                                )
                    for tt in range(NT):
                        nc.tensor.matmul(
                            pss[tt][:], ones1[:], b2_r[:, nf * 512:(nf + 1) * 512],
                            start=False, stop=True,
                        )
                        xs = x_t[tt][:, nf * 512:(nf + 1) * 512]
                        nc.vector.tensor_add(out=xs, in0=xs, in1=pss[tt][:])

            # ---- final LN (folded) + token-sharded lm_head ----
            hfT = emit_ln()
            for ch in range(NCH):
                wlm_c = big.tile([128, NCT, VCW], BF, tag="big", name="wlm_c")
                nc.sync.dma_start(
                    out=wlm_c[:],
                    in_=_ap(wlm_d[0:1, 0:1, 0:1, 0:1], ch * NCT * 128 * VCW,
                            [[VCW, 128], [128 * VCW, NCT], [1, VCW]]),
                )
                for th in range(2):
                    lg = lgout.tile([128, 2, VCW], BF, tag="lg", name="lg")
                    for ti in range(2):
                        tt = th * 2 + ti
                        ps = ps_acc.tile([128, VCW], F32, tag="acc", name="acc")
                        for ct in range(NCT):
                            nc.tensor.matmul(
                                ps[:], hfT[:, ct, tt * 128:(tt + 1) * 128], wlm_c[:, ct, :],
                                start=(ct == 0), stop=(ct == NCT - 1),
                            )
                        if ti % 2 == 0:
                            nc.vector.tensor_copy(out=lg[:, ti, :], in_=ps[:])
                        else:
                            nc.scalar.activation(lg[:, ti, :], ps[:], AF.Copy)
                    nc.sync.dma_start(
                        out=_ap(logits_d[0:1, 0:1], th * 2 * 128 * V + ch * VCW,
                                [[V, 128], [128 * V, 2], [1, VCW]]),
                        in_=lg[:],
                    )

    nc.compile()
    _prog_cache[key] = nc
    return nc


def _prep_inputs(inputs):
    f = {k: np.asarray(v) for k, v in inputs.items()}
    idx = f["idx"].astype(np.int64)
    emb = f["emb"].astype(np.float32)
    pos = f["pos_enc"].astype(np.float32)
    x_full = emb[idx] + pos[None, :, :]          # [B,T,C] f32

    scale = HD ** -0.5
    g1 = f["ln1_g"].astype(np.float32)
    b1ln = f["ln1_b"].astype(np.float32)
    g2 = f["ln2_g"].astype(np.float32)
    b2ln = f["ln2_b"].astype(np.float32)
    gf = f["lnf_g"].astype(np.float32)
    bfln = f["lnf_b"].astype(np.float32)
    W1 = f["W1"].astype(np.float32)
    Wv = f["Wv"].astype(np.float32)
    Wo = f["Wo"].astype(np.float32)
    Wlm = f["Wlm"].astype(np.float32)

    bf = lambda a: np.ascontiguousarray(a, dtype=np.float32).astype(BF16NP)
    b1_f = f["b1"].astype(np.float32) + np.einsum("lc,lcf->lf", b2ln, W1)
    bo_f = f["bo"].astype(np.float32) + np.einsum(
        "ld,ldc->lc", np.einsum("lc,lcd->ld", b1ln, Wv), Wo
    )
    blm_f = f["blm"].astype(np.float32) + bfln @ Wlm

    wq_s = f["Wq"].astype(np.float32) * scale * g1[:, :, None]
    wk_s = f["Wk"].astype(np.float32) * g1[:, :, None]
    wv_s = Wv * g1[:, :, None]
    wlm_blocks = np.ascontiguousarray(
        (Wlm * gf[:, None]).reshape(NCT, 128, NCH, VCW).transpose(2, 0, 1, 3)
    ).astype(BF16NP)

    shared = {
        "w1": bf(W1 * g2[:, :, None]),
        "w2": bf(f["W2"]),
        "b1": b1_f.astype(np.float32),
        "bo": bf(bo_f),
        "wo": bf(Wo),
        "b2": bf(f["b2"]),
        "wlm": wlm_blocks,
        "maskd": np.triu(np.ones((128, 128), dtype=np.float32)).astype(BF16NP),
    }

    x_flat = np.ascontiguousarray(x_full.reshape(B * T, C), dtype=np.float32)
    in_maps = []
    for c in range(NCORES):
        hc = slice(c * 128, (c + 1) * 128)
        im = dict(shared)
        # core c owns flat tokens [c*512, (c+1)*512) — seq c//2, half c%2 —
        # so gathered-hT chunk r is exactly rank r's contiguous token block
        im["x0"] = x_flat[c * TL:(c + 1) * TL]
        im["wq"] = bf(wq_s[:, :, hc])
        im["wk"] = bf(wk_s[:, :, hc])
        im["wv"] = bf(wv_s[:, :, hc])
        in_maps.append(im)
    return in_maps, blm_f


def kernel(**inputs):
    nc = _build()
    in_maps, blm_f = _prep_inputs(inputs)
    res = run_bass_kernel_spmd(nc, in_maps, list(range(NCORES)))
    full = np.zeros((B * T, V), dtype=np.float32)
    for c in range(NCORES):
        full[c * TL:(c + 1) * TL, :] = np.asarray(
            res.results[c]["logits"], dtype=np.float32
        )
    full += blm_f[None, :]
    return full.reshape(B, T, V)


# revision 40
# speedup vs baseline: 1.1179x; 1.0326x over previous
"""GPT forward pass on 8 Trainium2 NeuronCores — v2 (head-sharded attention).

Trunk (LN/FFN/residual/lm_head) is token-parallel: core c owns token tile c
(128 tokens) of each of the 4 sequences. Attention is Megatron head-sharded:
core c owns global heads {2c, 2c+1} and computes full causal attention for
those heads over all 4096 tokens — every (head, seq) unit has the identical
causal block structure (q-tile qt needs qt+1 k-tiles), so the SPMD program
is uniform across cores while skipping all fully-masked blocks (36/64).

Per layer: LN1 -> AllGather h^T (1 MB payload) -> Q/K/V for my 2 heads over
all tokens (SBUF-resident, no DRAM round-trip) -> causal attention ->
row-parallel Wo partial -> bf16 ReduceScatter back to token owners ->
residual -> LN2 -> FFN (token-parallel). lm_head is token-sharded: each
core streams the full lnf-folded Wlm and emits bf16 logits for its 512
tokens; blm is added on the host.

LN gains are folded into consuming weights host-side (exact); ln2/lnf
biases into b1/blm (exact); projection bias b2 enters the PSUM via a K=1
ones-row matmul; bo is added after the ReduceScatter. All matmuls bf16
with fp32 PSUM; residual stream and softmax stats stay fp32. Softmax is
transposed-layout with denominators from a ones-column of V (no max
subtraction needed at these scales — matches the reference to ~5e-3).
"""

import os
import sys

for _p in ("/opt/trn_rl_repo",):
    if os.path.isdir(_p) and _p not in sys.path:
        sys.path.insert(0, _p)

import numpy as np
import ml_dtypes

BF16NP = ml_dtypes.bfloat16

import concourse.bass as bass
import concourse.mybir as mybir
import concourse.tile as tile
from concourse import bacc
from concourse.bass_utils import run_bass_kernel_spmd
from concourse.masks import make_identity

F32 = mybir.dt.float32
BF = mybir.dt.bfloat16
AF = mybir.ActivationFunctionType

V, C, T, H, L, B = 32000, 1024, 1024, 16, 4, 4
HD = C // H          # 64
FF = 4 * C           # 4096
NCORES = 8
TL = 512             # local tokens per core (4 seqs x 128)
TT = B * T // 1      # 4096 total tokens (seq-major: t = s*1024 + pos)
SEQ = B
NT = TL // 128       # 4 local t-tiles; tile tt = seq tt
NCT = C // 128       # 8 c-tiles
NFT = FF // 128      # 32 f-tiles
NTB = TT // 128      # 32 global t-blocks
NCH = 64             # vocab chunks
VCW = V // NCH       # 500
LN_EPS = 1e-5

_prog_cache = {}


def _ap(t, offset, pattern):
    return bass.AP(tensor=t.tensor if isinstance(t, bass.AP) else t, offset=offset, ap=pattern)


def _build(LL=L, sim=False):
    key = (LL, sim)
    if key in _prog_cache:
        return _prog_cache[key]

    nc = bacc.Bacc("TRN2", target_bir_lowering=False, debug=False, num_devices=NCORES)

    x0 = nc.dram_tensor("x0", [TL, C], F32, kind="ExternalInput")
    mask_d = nc.dram_tensor("maskd", [128, 128], BF, kind="ExternalInput")
    wq_d = nc.dram_tensor("wq", [L, C, 128], BF, kind="ExternalInput")   # my head cols
    wk_d = nc.dram_tensor("wk", [L, C, 128], BF, kind="ExternalInput")
    wv_d = nc.dram_tensor("wv", [L, C, 128], BF, kind="ExternalInput")
    wo_d = nc.dram_tensor("wo", [L, C, C], BF, kind="ExternalInput")
    w1_d = nc.dram_tensor("w1", [L, C, FF], BF, kind="ExternalInput")
    w2_d = nc.dram_tensor("w2", [L, FF, C], BF, kind="ExternalInput")
    b1_d = nc.dram_tensor("b1", [L, FF], F32, kind="ExternalInput")
    bo_d = nc.dram_tensor("bo", [L, C], BF, kind="ExternalInput")
    b2_d = nc.dram_tensor("b2", [L, C], BF, kind="ExternalInput")
    wlm_d = nc.dram_tensor("wlm", [NCH, NCT, 128, VCW], BF, kind="ExternalInput")

    logits_d = nc.dram_tensor("logits", [TL, V], BF, kind="ExternalOutput")

    HTSZ = C * TL            # elems in one core's hT payload (1 MB bf16)

    with tile.TileContext(nc) as tc:
        import contextlib

        with contextlib.ExitStack() as ctx:
            # SBUF pools (~per-partition KB)
            const = ctx.enter_context(tc.tile_pool(name="const", bufs=1))      # .6
            xpool = ctx.enter_context(tc.tile_pool(name="x", bufs=1))          # 16
            hpool = ctx.enter_context(tc.tile_pool(name="h", bufs=5))          # 10
            tpool = ctx.enter_context(tc.tile_pool(name="hT", bufs=1))         # 8
            big = ctx.enter_context(tc.tile_pool(name="big", bufs=2))          # 64
            qkt = ctx.enter_context(tc.tile_pool(name="qkt", bufs=1))          # 16
            vsb_p = ctx.enter_context(tc.tile_pool(name="vsb", bufs=1))        # 8.3
            otm_p = ctx.enter_context(tc.tile_pool(name="otm", bufs=1))        # 8
            w4 = ctx.enter_context(tc.tile_pool(name="w4", bufs=5))            # 20
            qwp = ctx.enter_context(tc.tile_pool(name="qw", bufs=1))           # 6
            gbpool = ctx.enter_context(tc.tile_pool(name="gb", bufs=1))        # 4.5
            misc = ctx.enter_context(tc.tile_pool(name="misc", bufs=2))        # .6
            pt_pool = ctx.enter_context(tc.tile_pool(name="pt", bufs=1))       # 9
            oraw_pool = ctx.enter_context(tc.tile_pool(name="oraw", bufs=2))   # 8
            os_pool = ctx.enter_context(tc.tile_pool(name="oS", bufs=2))       # 4
            rcp_pool = ctx.enter_context(tc.tile_pool(name="rcp", bufs=1))     # 4
            rb_pool = ctx.enter_context(tc.tile_pool(name="rb", bufs=2))       # 8
            pd_pool = ctx.enter_context(tc.tile_pool(name="pd", bufs=3))       # 6
            rcv_pool = ctx.enter_context(tc.tile_pool(name="rcv", bufs=2))     # 4
            lgout = ctx.enter_context(tc.tile_pool(name="lgout", bufs=2))      # 8
            ps_acc = ctx.enter_context(tc.tile_pool(name="psacc", bufs=4, space="PSUM"))
            ps_st = ctx.enter_context(tc.tile_pool(name="psst", bufs=2, space="PSUM"))
            ps_ov = ctx.enter_context(tc.tile_pool(name="psov", bufs=2, space="PSUM"))
            dram = ctx.enter_context(tc.tile_pool(name="dram", bufs=1, space="DRAM"))

            ident = const.tile([128, 128], BF, name="ident")
            make_identity(nc, ident)
            eps_t = const.tile([128, 1], F32, name="eps")
            nc.vector.memset(eps_t[:], LN_EPS)
            mask_t = const.tile([128, 128], BF, name="mask")
            nc.sync.dma_start(out=mask_t[:], in_=mask_d[:])
            ones1 = const.tile([1, 128], BF, name="ones1")
            nc.vector.memset(ones1[:], 1.0)

            hT_loc = dram.tile([HTSZ], BF, name="hT_loc")
            o_loc = dram.tile([NCORES * 128 * 512], BF, name="o_loc")

            # persistent residual stream fp32: tile tt = seq tt
            x_t = [xpool.tile([128, C], F32, tag=f"x{tt}", name=f"x{tt}") for tt in range(NT)]
            for tt in range(NT):
                nc.sync.dma_start(out=x_t[tt][:], in_=x0[tt * 128:(tt + 1) * 128, :])

            def emit_ln():
                """x_t -> (x-m)*rstd bf16, transposed hT tiles (g/b folded away)."""
                h_tiles = []
                for tt in range(NT):
                    stats = misc.tile([128, 2, 6], F32, name="stats", tag="stats")
                    xv = x_t[tt][:].rearrange("p (s d) -> p s d", s=2)
                    nc.vector.bn_stats(out=stats[:, 0, :], in_=xv[:, 0, :])
                    nc.vector.bn_stats(out=stats[:, 1, :], in_=xv[:, 1, :])
                    mv = misc.tile([128, 2], F32, name="mv", tag="mv")
                    nc.vector.bn_aggr(out=mv[:], in_=stats[:])
                    rstd = misc.tile([128, 1], F32, name="rstd", tag="rstd")
                    nc.scalar.activation(rstd[:], mv[:, 1:2], AF.Sqrt, bias=eps_t[:])
                    nc.vector.reciprocal(rstd[:], rstd[:])
                    h = hpool.tile([128, C], BF, tag="h", name="h")
                    nc.vector.tensor_scalar(
                        out=h[:], in0=x_t[tt][:], scalar1=mv[:, 0:1], scalar2=rstd[:],
                        op0=mybir.AluOpType.subtract, op1=mybir.AluOpType.mult,
                    )
                    h_tiles.append(h)
                hT_all = tpool.tile([128, NCT, 512], BF, tag="hTall", name="hTall")
                for ct in range(NCT):
                    pst = ps_st.tile([128, 512], BF, tag="st", name="pst")
                    for tt in range(NT):
                        nc.tensor.transpose(
                            pst[:, tt * 128:(tt + 1) * 128],
                            h_tiles[tt][:, ct * 128:(ct + 1) * 128],
                            ident[:],
                        )
                    nc.vector.tensor_copy(out=hT_all[:, ct, :], in_=pst[:])
                return hT_all

            for l in range(LL):
                lw = l % L
                # per-layer bias/const tiles (issued early; Pool queue quiet)
                b2_r = gbpool.tile([1, C], BF, tag="b2r", name="b2_r")
                nc.sync.dma_start(out=b2_r[:], in_=b2_d[lw:lw + 1, :])
                bo_r = gbpool.tile([1, C], BF, tag="bor", name="bo_r")
                nc.sync.dma_start(out=bo_r[:], in_=bo_d[lw:lw + 1, :])
                b1_t = misc.tile([128, NFT], F32, tag="b1", name="b1_t")
                nc.gpsimd.dma_start(
                    out=b1_t[:], in_=_ap(b1_d, lw * FF, [[1, 128], [128, NFT]])
                )
                # my-head projection weights [128c-in-ct, ct, 128d] — one DMA each
                wq_t = qwp.tile([128, NCT, 128], BF, tag="qw", name="wq_t")
                wk_t = qwp.tile([128, NCT, 128], BF, tag="kw", name="wk_t")
                wv_t = qwp.tile([128, NCT, 128], BF, tag="vw", name="wv_t")
                for wt, wd in ((wq_t, wq_d), (wk_t, wk_d), (wv_t, wv_d)):
                    nc.sync.dma_start(
                        out=wt[:],
                        in_=_ap(wd[0:1, 0:1, 0:1], lw * C * 128,
                                [[128, 128], [128 * 128, NCT], [1, 128]]),
                    )

                # ---- LN1 -> hT, publish (one DMA), AllGather ----
                hT = emit_ln()
                nc.sync.dma_start(
                    out=_ap(hT_loc, 0, [[512, 128], [128 * 512, NCT], [1, 512]]),
                    in_=hT[:],
                )
                hT_full = dram.tile(
                    [NCORES * HTSZ], BF,
                    addr_space="Local" if sim else "Shared", name=f"hT_full{l}",
                )
                if sim:
                    nc.sync.dma_start(
                        out=_ap(hT_full, 0, [[2048, HTSZ // 2048], [1, 2048]]),
                        in_=_ap(hT_loc, 0, [[2048, HTSZ // 2048], [1, 2048]]),
                    )
                else:
                    nc.gpsimd.collective_compute(
                        "AllGather",
                        mybir.AluOpType.bypass,
                        replica_groups=[list(range(NCORES))],
                        ins=[_ap(hT_loc, 0, [[2048, HTSZ // 2048], [1, 2048]])],
                        outs=[_ap(hT_full, 0, [[2048, NCORES * HTSZ // 2048], [1, 2048]])],
                    )

                # gathered h^T lands per rank-chunk so QKV(seq s) starts after
                # 2 chunks, and attention(s) overlaps QKV(s+1)
                hT_sb = [None, None]

                def load_chunk(ch):
                    half = ch // 4
                    if hT_sb[half] is None:
                        hT_sb[half] = big.tile(
                            [128, 4, NCT, 512], BF, tag="big", name=f"hTsb{half}"
                        )
                    nc.sync.dma_start(
                        out=hT_sb[half][:, ch % 4, :, :],
                        in_=_ap(hT_full, ch * HTSZ, [[512, 128], [128 * 512, NCT], [1, 512]]),
                    )

                def htf(ct, ch):
                    return hT_sb[ch // 4][:, ch % 4, ct, :]

                qT_s, kT_s, v_ss, oTm_s = [], [], [], []
                for s in range(SEQ):
                    qT_s.append(qkt.tile([128, 1024], BF, tag=f"qT{s}", name=f"qT{s}"))
                    kT_s.append(qkt.tile([128, 1024], BF, tag=f"kT{s}", name=f"kT{s}"))
                    v_ss.append(vsb_p.tile([128, 8, 2, HD + 1], BF, tag=f"v{s}", name=f"v{s}"))
                    oTm_s.append(otm_p.tile([128, 1024], BF, tag=f"oTm{s}", name=f"oTm{s}"))

                for s in range(SEQ):
                    for hh in range(2):
                        load_chunk(2 * s + hh)
                    # K^T then Q^T for this seq (2 chunks each)
                    for dst, wt_l in ((kT_s[s], wk_t), (qT_s[s], wq_t)):
                        for hh in range(2):
                            ch = 2 * s + hh
                            ps = ps_acc.tile([128, 512], F32, tag="acc", name="acc")
                            for ct in range(NCT):
                                nc.tensor.matmul(
                                    ps[:], wt_l[:, ct, :], htf(ct, ch),
                                    start=(ct == 0), stop=(ct == NCT - 1),
                                )
                            nc.vector.tensor_copy(
                                out=dst[:, hh * 512:(hh + 1) * 512], in_=ps[:]
                            )
                    # V natural [t, my 128 d] with ones column
                    v_sb = v_ss[s]
                    nc.vector.memset(v_sb[:, :, :, HD:HD + 1], 1.0)
                    for i in range(8):
                        tb = s * 8 + i
                        ps = ps_acc.tile([128, 128], F32, tag="acc", name="psv")
                        for ct in range(NCT):
                            nc.tensor.matmul(
                                ps[:], htf(ct, tb // 4)[:, (tb % 4) * 128:(tb % 4 + 1) * 128],
                                wv_t[:, ct, :],
                                start=(ct == 0), stop=(ct == NCT - 1),
                            )
                        nc.vector.tensor_copy(out=v_sb[:, i, 0, 0:HD], in_=ps[:, 0:HD])
                        nc.vector.tensor_copy(out=v_sb[:, i, 1, 0:HD], in_=ps[:, HD:2 * HD])

                    # ---- causal attention: kt-major QK/exp; PV with pT
                    # stationary and V moving (N=65), so output is q-major and
                    # the softmax denominator is a per-partition scalar ----
                    OFF = [128 * (8 * i - i * (i - 1) // 2) for i in range(8)]

                    for hp in range(2):
                        poff = hp * HD
                        pTa = pt_pool.tile([128, 4608], BF, tag="pt", name="pt")

                        onat_all = os_pool.tile([128, 8, HD], BF, tag="onat", name="onat")

                        def emit_pv(qt):
                            ov = ps_ov.tile([128, 128], F32, tag="ov", name="ov")
                            for k2 in range(qt + 1):
                                nc.tensor.matmul(
                                    ov[:, 0:HD + 1],
                                    pTa[:, OFF[k2] + (qt - k2) * 128:OFF[k2] + (qt - k2 + 1) * 128],
                                    v_sb[:, k2, hp, :],
                                    start=(k2 == 0), stop=(k2 == qt),
                                )
                            rec = misc.tile([128, 1], F32, tag="rec", name="rec")
                            nc.vector.reciprocal(rec[:], ov[:, HD:HD + 1])
                            nc.vector.tensor_scalar(
                                out=onat_all[:, qt, :], in0=ov[:, 0:HD], scalar1=rec[:],
                                scalar2=None, op0=mybir.AluOpType.mult,
                            )

                        for kt in range(8):
                            nq = 8 - kt
                            off = 0
                            while off < nq * 128:
                                cols = min(512, nq * 128 - off)
                                st = ps_st.tile([128, 512], F32, tag="st", name="st")
                                nc.tensor.matmul(
                                    st[:, 0:cols],
                                    kT_s[s][poff:poff + HD, kt * 128:(kt + 1) * 128],
                                    qT_s[s][poff:poff + HD, kt * 128 + off:kt * 128 + off + cols],
                                    start=True, stop=True,
                                )
                                nc.scalar.activation(
                                    pTa[:, OFF[kt] + off:OFF[kt] + off + cols], st[:, 0:cols], AF.Exp
                                )
                                off += cols
                            nc.vector.tensor_mul(
                                out=pTa[:, OFF[kt]:OFF[kt] + 128],
                                in0=pTa[:, OFF[kt]:OFF[kt] + 128], in1=mask_t[:],
                            )
                            if kt >= 1:
                                emit_pv(kt - 1)
                        emit_pv(6)
                        emit_pv(7)
                        for qp in range(2):
                            pst = ps_st.tile([128, 512], BF, tag="st", name="ot")
                            for qi in range(4):
                                qt = qp * 4 + qi
                                nc.tensor.transpose(
                                    pst[0:HD, qi * 128:(qi + 1) * 128],
                                    onat_all[:, qt, :], ident[:],
                                )
                            nc.vector.tensor_copy(
                                out=oTm_s[s][poff:poff + HD, qp * 512:(qp + 1) * 512],
                                in_=pst[0:HD, :],
                            )
                    nc.sync.dma_start(
                        out=_ap(o_loc, 2 * s * 128 * 512, [[512, 128], [128 * 512, 2], [1, 512]]),
                        in_=oTm_s[s][:],
                    )
                o_recv = dram.tile([NCORES * 128 * 512], BF, name=f"orecv{l}")
                if sim:
                    nc.sync.dma_start(
                        out=_ap(o_recv, 0, [[2048, NCORES * 128 * 512 // 2048], [1, 2048]]),
                        in_=_ap(o_loc, 0, [[2048, NCORES * 128 * 512 // 2048], [1, 2048]]),
                    )
                else:
                    nc.gpsimd.collective_compute(
                        "AllToAll",
                        mybir.AluOpType.bypass,
                        replica_groups=[list(range(NCORES))],
                        ins=[_ap(o_loc, 0, [[2048, NCORES * 128 * 512 // 2048], [1, 2048]])],
                        outs=[_ap(o_recv, 0, [[2048, NCORES * 128 * 512 // 2048], [1, 2048]])],
                    )
                orv = [otm_p.tile([128, 4, 512], BF, tag=f"orv{h_}", name="orv")
                       for h_ in range(2)]
                for h_ in range(2):
                    nc.sync.dma_start(
                        out=orv[h_][:],
                        in_=_ap(o_recv, h_ * 4 * 128 * 512,
                                [[512, 128], [128 * 512, 4], [1, 512]]),
                    )
                wo_ts = []
                for nf in range(2):
                    for cg in range(2):
                        wt = w4.tile([128, 4, 512], BF, tag="w", name="wot")
                        nc.sync.dma_start(
                            out=wt[:],
                            in_=_ap(wo_d[0:1, 0:1, 0:1],
                                    lw * C * C + cg * 512 * C + nf * 512,
                                    [[C, 128], [128 * C, 4], [1, 512]]),
                        )
                        wo_ts.append(wt)
                # tt-outer Wo + residual, LN2 stats fused per tile
                h2_tiles = []
                for tt in range(NT):
                    for nf in range(2):
                        ps = ps_acc.tile([128, 512], F32, tag="acc", name="acc")
                        for cg in range(2):
                            wt = wo_ts[nf * 2 + cg]
                            for ci in range(4):
                                nc.tensor.matmul(
                                    ps[:], orv[cg][:, ci, tt * 128:(tt + 1) * 128], wt[:, ci, :],
                                    start=(cg == 0 and ci == 0), stop=False,
                                )
                        nc.tensor.matmul(
                            ps[:], ones1[:], bo_r[:, nf * 512:(nf + 1) * 512],
                            start=False, stop=True,
                        )
                        xs = x_t[tt][:, nf * 512:(nf + 1) * 512]
                        nc.vector.tensor_add(out=xs, in0=xs, in1=ps[:])
                    stats = misc.tile([128, 2, 6], F32, name="stats", tag="stats")
                    xv = x_t[tt][:].rearrange("p (s d) -> p s d", s=2)
                    nc.vector.bn_stats(out=stats[:, 0, :], in_=xv[:, 0, :])
                    nc.vector.bn_stats(out=stats[:, 1, :], in_=xv[:, 1, :])
                    mv = misc.tile([128, 2], F32, name="mv", tag="mv")
                    nc.vector.bn_aggr(out=mv[:], in_=stats[:])
                    rstd = misc.tile([128, 1], F32, name="rstd", tag="rstd")
                    nc.scalar.activation(rstd[:], mv[:, 1:2], AF.Sqrt, bias=eps_t[:])
                    nc.vector.reciprocal(rstd[:], rstd[:])
                    h = hpool.tile([128, C], BF, tag="h", name="h")
                    nc.vector.tensor_scalar(
                        out=h[:], in0=x_t[tt][:], scalar1=mv[:, 0:1], scalar2=rstd[:],
                        op0=mybir.AluOpType.subtract, op1=mybir.AluOpType.mult,
                    )
                    h2_tiles.append(h)
                h2T = tpool.tile([128, NCT, 512], BF, tag="hTall", name="h2Tall")
                for ct in range(NCT):
                    pst = ps_st.tile([128, 512], BF, tag="st", name="pst")
                    for tt in range(NT):
                        nc.tensor.transpose(
                            pst[:, tt * 128:(tt + 1) * 128],
                            h2_tiles[tt][:, ct * 128:(ct + 1) * 128],
                            ident[:],
                        )
                    nc.vector.tensor_copy(out=h2T[:, ct, :], in_=pst[:])

                # ---- FFN (token-parallel, biases in-psum / in-activation) ----
                ug = big.tile([128, NFT, 512], BF, tag="big", name="ug")
                for fg in range(8):
                    pss = [ps_acc.tile([128, 512], F32, tag="acc", name="acc") for _ in range(4)]
                    for hf in range(2):
                        wt = w4.tile([128, 4, 512], BF, tag="w", name="w1t")
                        nc.sync.dma_start(
                            out=wt[:],
                            in_=_ap(w1_d[0:1, 0:1, 0:1],
                                    lw * C * FF + hf * 512 * FF + fg * 512,
                                    [[FF, 128], [128 * FF, 4], [1, 512]]),
                        )
                        for ci in range(4):
                            ct = hf * 4 + ci
                            for f4 in range(4):
                                nc.tensor.matmul(
                                    pss[f4][:], wt[:, ci, f4 * 128:(f4 + 1) * 128], h2T[:, ct, :],
                                    start=(ct == 0), stop=(ct == NCT - 1),
                                )
                    for f4 in range(4):
                        ft = fg * 4 + f4
                        nc.scalar.activation(
                            ug[:, ft, :], pss[f4][:], AF.Gelu, bias=b1_t[:, ft:ft + 1]
                        )
                for nf in range(2):
                    pss = [ps_acc.tile([128, 512], F32, tag="acc", name="acc") for _ in range(4)]
                    for g8 in range(8):
                        wt = w4.tile([128, 4, 512], BF, tag="w", name="w2t")
                        nc.sync.dma_start(
                            out=wt[:],
                            in_=_ap(w2_d[0:1, 0:1, 0:1],
                                    lw * FF * C + g8 * 512 * C + nf * 512,
                                    [[C, 128], [128 * C, 4], [1, 512]]),
                        )
                        for fi in range(4):
                            ft = g8 * 4 + fi
                            for tt in range(NT):
                                nc.tensor.matmul(
                                    pss[tt][:], ug[:, ft, tt * 128:(tt + 1) * 128], wt[:, fi, :],
                                    start=(ft == 0), stop=False,
                                )
                    for tt in range(NT):
                        nc.tensor.matmul(
                            pss[tt][:], ones1[:], b2_r[:, nf * 512:(nf + 1) * 512],
                            start=False, stop=True,
                        )
                        xs = x_t[tt][:, nf * 512:(nf + 1) * 512]
                        nc.vector.tensor_add(out=xs, in0=xs, in1=pss[tt][:])

            # ---- final LN (folded) + token-sharded lm_head ----
            hfT = emit_ln()
            for ch in range(NCH):
                wlm_c = big.tile([128, NCT, VCW], BF, tag="big", name="wlm_c")
                nc.sync.dma_start(
                    out=wlm_c[:],
                    in_=_ap(wlm_d[0:1, 0:1, 0:1, 0:1], ch * NCT * 128 * VCW,
                            [[VCW, 128], [128 * VCW, NCT], [1, VCW]]),
                )
                for th in range(2):
                    lg = lgout.tile([128, 2, VCW], BF, tag="lg", name="lg")
                    for ti in range(2):
                        tt = th * 2 + ti
                        ps = ps_acc.tile([128, VCW], F32, tag="acc", name="acc")
                        for ct in range(NCT):
                            nc.tensor.matmul(
                                ps[:], hfT[:, ct, tt * 128:(tt + 1) * 128], wlm_c[:, ct, :],
                                start=(ct == 0), stop=(ct == NCT - 1),
                            )
                        if ti % 2 == 0:
                            nc.vector.tensor_copy(out=lg[:, ti, :], in_=ps[:])
                        else:
                            nc.scalar.activation(lg[:, ti, :], ps[:], AF.Copy)
                    nc.sync.dma_start(
                        out=_ap(logits_d[0:1, 0:1], th * 2 * 128 * V + ch * VCW,
                                [[V, 128], [128 * V, 2], [1, VCW]]),
                        in_=lg[:],
                    )

    nc.compile()
    _prog_cache[key] = nc
    return nc


def _prep_inputs(inputs):
    f = {k: np.asarray(v) for k, v in inputs.items()}
    idx = f["idx"].astype(np.int64)
    emb = f["emb"].astype(np.float32)
    pos = f["pos_enc"].astype(np.float32)
    x_full = emb[idx] + pos[None, :, :]          # [B,T,C] f32

    scale = HD ** -0.5
    g1 = f["ln1_g"].astype(np.float32)
    b1ln = f["ln1_b"].astype(np.float32)
    g2 = f["ln2_g"].astype(np.float32)
    b2ln = f["ln2_b"].astype(np.float32)
    gf = f["lnf_g"].astype(np.float32)
    bfln = f["lnf_b"].astype(np.float32)
    W1 = f["W1"].astype(np.float32)
    Wv = f["Wv"].astype(np.float32)
    Wo = f["Wo"].astype(np.float32)
    Wlm = f["Wlm"].astype(np.float32)

    bf = lambda a: np.ascontiguousarray(a, dtype=np.float32).astype(BF16NP)
    b1_f = f["b1"].astype(np.float32) + np.einsum("lc,lcf->lf", b2ln, W1)
    bo_f = f["bo"].astype(np.float32) + np.einsum(
        "ld,ldc->lc", np.einsum("lc,lcd->ld", b1ln, Wv), Wo
    )
    blm_f = f["blm"].astype(np.float32) + bfln @ Wlm

    wq_s = f["Wq"].astype(np.float32) * scale * g1[:, :, None]
    wk_s = f["Wk"].astype(np.float32) * g1[:, :, None]
    wv_s = Wv * g1[:, :, None]
    wlm_blocks = np.ascontiguousarray(
        (Wlm * gf[:, None]).reshape(NCT, 128, NCH, VCW).transpose(2, 0, 1, 3)
    ).astype(BF16NP)

    shared = {
        "w1": bf(W1 * g2[:, :, None]),
        "w2": bf(f["W2"]),
        "b1": b1_f.astype(np.float32),
        "bo": bf(bo_f),
        "wo": bf(Wo),
        "b2": bf(f["b2"]),
        "wlm": wlm_blocks,
        "maskd": np.triu(np.ones((128, 128), dtype=np.float32)).astype(BF16NP),
    }

    x_flat = np.ascontiguousarray(x_full.reshape(B * T, C), dtype=np.float32)
    in_maps = []
    for c in range(NCORES):
        hc = slice(c * 128, (c + 1) * 128)
        im = dict(shared)
        # core c owns flat tokens [c*512, (c+1)*512) — seq c//2, half c%2 —
        # so gathered-hT chunk r is exactly rank r's contiguous token block
        im["x0"] = x_flat[c * TL:(c + 1) * TL]
        im["wq"] = bf(wq_s[:, :, hc])
        im["wk"] = bf(wk_s[:, :, hc])
        im["wv"] = bf(wv_s[:, :, hc])
        in_maps.append(im)
    return in_maps, blm_f


def kernel(**inputs):
    nc = _build()
    in_maps, blm_f = _prep_inputs(inputs)
    res = run_bass_kernel_spmd(nc, in_maps, list(range(NCORES)))
    full = np.zeros((B * T, V), dtype=np.float32)
    for c in range(NCORES):
        full[c * TL:(c + 1) * TL, :] = np.asarray(
            res.results[c]["logits"], dtype=np.float32
        )
    full += blm_f[None, :]
    return full.reshape(B, T, V)


# revision 42
# speedup vs baseline: 1.1217x; 1.0034x over previous
"""GPT forward pass on 8 Trainium2 NeuronCores — v2 (head-sharded attention).

Trunk (LN/FFN/residual/lm_head) is token-parallel: core c owns token tile c
(128 tokens) of each of the 4 sequences. Attention is Megatron head-sharded:
core c owns global heads {2c, 2c+1} and computes full causal attention for
those heads over all 4096 tokens — every (head, seq) unit has the identical
causal block structure (q-tile qt needs qt+1 k-tiles), so the SPMD program
is uniform across cores while skipping all fully-masked blocks (36/64).

Per layer: LN1 -> AllGather h^T (1 MB payload) -> Q/K/V for my 2 heads over
all tokens (SBUF-resident, no DRAM round-trip) -> causal attention ->
row-parallel Wo partial -> bf16 ReduceScatter back to token owners ->
residual -> LN2 -> FFN (token-parallel). lm_head is token-sharded: each
core streams the full lnf-folded Wlm and emits bf16 logits for its 512
tokens; blm is added on the host.

LN gains are folded into consuming weights host-side (exact); ln2/lnf
biases into b1/blm (exact); projection bias b2 enters the PSUM via a K=1
ones-row matmul; bo is added after the ReduceScatter. All matmuls bf16
with fp32 PSUM; residual stream and softmax stats stay fp32. Softmax is
transposed-layout with denominators from a ones-column of V (no max
subtraction needed at these scales — matches the reference to ~5e-3).
"""

import os
import sys

for _p in ("/opt/trn_rl_repo",):
    if os.path.isdir(_p) and _p not in sys.path:
        sys.path.insert(0, _p)

import numpy as np
import ml_dtypes

BF16NP = ml_dtypes.bfloat16

import concourse.bass as bass
import concourse.mybir as mybir
import concourse.tile as tile
from concourse import bacc
from concourse.bass_utils import run_bass_kernel_spmd
from concourse.masks import make_identity

F32 = mybir.dt.float32
BF = mybir.dt.bfloat16
AF = mybir.ActivationFunctionType

V, C, T, H, L, B = 32000, 1024, 1024, 16, 4, 4
HD = C // H          # 64
FF = 4 * C           # 4096
NCORES = 8
TL = 512             # local tokens per core (4 seqs x 128)
TT = B * T // 1      # 4096 total tokens (seq-major: t = s*1024 + pos)
SEQ = B
NT = TL // 128       # 4 local t-tiles; tile tt = seq tt
NCT = C // 128       # 8 c-tiles
NFT = FF // 128      # 32 f-tiles
NTB = TT // 128      # 32 global t-blocks
NCH = 64             # vocab chunks
VCW = V // NCH       # 500
LN_EPS = 1e-5

_prog_cache = {}


def _ap(t, offset, pattern):
    return bass.AP(tensor=t.tensor if isinstance(t, bass.AP) else t, offset=offset, ap=pattern)


def _build(LL=L, sim=False):
    key = (LL, sim)
    if key in _prog_cache:
        return _prog_cache[key]

    nc = bacc.Bacc("TRN2", target_bir_lowering=False, debug=False, num_devices=NCORES)

    x0 = nc.dram_tensor("x0", [TL, C], F32, kind="ExternalInput")
    mask_d = nc.dram_tensor("maskd", [128, 128], BF, kind="ExternalInput")
    wq_d = nc.dram_tensor("wq", [L, C, 128], BF, kind="ExternalInput")   # my head cols
    wk_d = nc.dram_tensor("wk", [L, C, 128], BF, kind="ExternalInput")
    wv_d = nc.dram_tensor("wv", [L, C, 128], BF, kind="ExternalInput")
    wo_d = nc.dram_tensor("wo", [L, C, C], BF, kind="ExternalInput")
    w1_d = nc.dram_tensor("w1", [L, C, FF], BF, kind="ExternalInput")
    w2_d = nc.dram_tensor("w2", [L, FF, C], BF, kind="ExternalInput")
    b1_d = nc.dram_tensor("b1", [L, FF], F32, kind="ExternalInput")
    bo_d = nc.dram_tensor("bo", [L, C], BF, kind="ExternalInput")
    b2_d = nc.dram_tensor("b2", [L, C], BF, kind="ExternalInput")
    wlm_d = nc.dram_tensor("wlm", [NCH, NCT, 128, VCW], BF, kind="ExternalInput")

    logits_d = nc.dram_tensor("logits", [TL, V], BF, kind="ExternalOutput")

    HTSZ = C * TL            # elems in one core's hT payload (1 MB bf16)

    with tile.TileContext(nc) as tc:
        import contextlib

        with contextlib.ExitStack() as ctx:
            # SBUF pools (~per-partition KB)
            const = ctx.enter_context(tc.tile_pool(name="const", bufs=1))      # .6
            xpool = ctx.enter_context(tc.tile_pool(name="x", bufs=1))          # 16
            hpool = ctx.enter_context(tc.tile_pool(name="h", bufs=5))          # 10
            tpool = ctx.enter_context(tc.tile_pool(name="hT", bufs=1))         # 8
            big = ctx.enter_context(tc.tile_pool(name="big", bufs=2))          # 64
            qkt = ctx.enter_context(tc.tile_pool(name="qkt", bufs=1))          # 16
            vsb_p = ctx.enter_context(tc.tile_pool(name="vsb", bufs=1))        # 8.3
            otm_p = ctx.enter_context(tc.tile_pool(name="otm", bufs=1))        # 8
            w4 = ctx.enter_context(tc.tile_pool(name="w4", bufs=5))            # 20
            qwp = ctx.enter_context(tc.tile_pool(name="qw", bufs=1))           # 6
            gbpool = ctx.enter_context(tc.tile_pool(name="gb", bufs=1))        # 4.5
            misc = ctx.enter_context(tc.tile_pool(name="misc", bufs=2))        # .6
            pt_pool = ctx.enter_context(tc.tile_pool(name="pt", bufs=2))       # 18
            os_pool = ctx.enter_context(tc.tile_pool(name="oS", bufs=2))       # 4
            rcp_pool = ctx.enter_context(tc.tile_pool(name="rcp", bufs=1))     # 4
            pd_pool = ctx.enter_context(tc.tile_pool(name="pd", bufs=3))       # 6
            rcv_pool = ctx.enter_context(tc.tile_pool(name="rcv", bufs=2))     # 4
            lgout = ctx.enter_context(tc.tile_pool(name="lgout", bufs=2))      # 8
            ps_acc = ctx.enter_context(tc.tile_pool(name="psacc", bufs=4, space="PSUM"))
            ps_st = ctx.enter_context(tc.tile_pool(name="psst", bufs=2, space="PSUM"))
            ps_ov = ctx.enter_context(tc.tile_pool(name="psov", bufs=2, space="PSUM"))
            dram = ctx.enter_context(tc.tile_pool(name="dram", bufs=1, space="DRAM"))

            ident = const.tile([128, 128], BF, name="ident")
            make_identity(nc, ident)
            eps_t = const.tile([128, 1], F32, name="eps")
            nc.vector.memset(eps_t[:], LN_EPS)
            mask_t = const.tile([128, 128], BF, name="mask")
            nc.sync.dma_start(out=mask_t[:], in_=mask_d[:])
            ones1 = const.tile([1, 128], BF, name="ones1")
            nc.vector.memset(ones1[:], 1.0)

            hT_loc = dram.tile([HTSZ], BF, name="hT_loc")
            o_loc = dram.tile([NCORES * 128 * 512], BF, name="o_loc")

            # persistent residual stream fp32: tile tt = seq tt
            x_t = [xpool.tile([128, C], F32, tag=f"x{tt}", name=f"x{tt}") for tt in range(NT)]
            for tt in range(NT):
                nc.sync.dma_start(out=x_t[tt][:], in_=x0[tt * 128:(tt + 1) * 128, :])

            def emit_ln():
                """x_t -> (x-m)*rstd bf16, transposed hT tiles (g/b folded away)."""
                h_tiles = []
                for tt in range(NT):
                    stats = misc.tile([128, 2, 6], F32, name="stats", tag="stats")
                    xv = x_t[tt][:].rearrange("p (s d) -> p s d", s=2)
                    nc.vector.bn_stats(out=stats[:, 0, :], in_=xv[:, 0, :])
                    nc.vector.bn_stats(out=stats[:, 1, :], in_=xv[:, 1, :])
                    mv = misc.tile([128, 2], F32, name="mv", tag="mv")
                    nc.vector.bn_aggr(out=mv[:], in_=stats[:])
                    rstd = misc.tile([128, 1], F32, name="rstd", tag="rstd")
                    nc.scalar.activation(rstd[:], mv[:, 1:2], AF.Sqrt, bias=eps_t[:])
                    nc.vector.reciprocal(rstd[:], rstd[:])
                    h = hpool.tile([128, C], BF, tag="h", name="h")
                    nc.vector.tensor_scalar(
                        out=h[:], in0=x_t[tt][:], scalar1=mv[:, 0:1], scalar2=rstd[:],
                        op0=mybir.AluOpType.subtract, op1=mybir.AluOpType.mult,
                    )
                    h_tiles.append(h)
                hT_all = tpool.tile([128, NCT, 512], BF, tag="hTall", name="hTall")
                for ct in range(NCT):
                    pst = ps_st.tile([128, 512], BF, tag="st", name="pst")
                    for tt in range(NT):
                        nc.tensor.transpose(
                            pst[:, tt * 128:(tt + 1) * 128],
                            h_tiles[tt][:, ct * 128:(ct + 1) * 128],
                            ident[:],
                        )
                    nc.vector.tensor_copy(out=hT_all[:, ct, :], in_=pst[:])
                return hT_all

            for l in range(LL):
                lw = l % L
                # per-layer bias/const tiles (issued early; Pool queue quiet)
                b2_r = gbpool.tile([1, C], BF, tag="b2r", name="b2_r")
                nc.sync.dma_start(out=b2_r[:], in_=b2_d[lw:lw + 1, :])
                bo_r = gbpool.tile([1, C], BF, tag="bor", name="bo_r")
                nc.sync.dma_start(out=bo_r[:], in_=bo_d[lw:lw + 1, :])
                b1_t = misc.tile([128, NFT], F32, tag="b1", name="b1_t")
                nc.gpsimd.dma_start(
                    out=b1_t[:], in_=_ap(b1_d, lw * FF, [[1, 128], [128, NFT]])
                )
                # my-head projection weights [128c-in-ct, ct, 128d] — one DMA each
                wq_t = qwp.tile([128, NCT, 128], BF, tag="qw", name="wq_t")
                wk_t = qwp.tile([128, NCT, 128], BF, tag="kw", name="wk_t")
                wv_t = qwp.tile([128, NCT, 128], BF, tag="vw", name="wv_t")
                for wt, wd in ((wq_t, wq_d), (wk_t, wk_d), (wv_t, wv_d)):
                    nc.sync.dma_start(
                        out=wt[:],
                        in_=_ap(wd[0:1, 0:1, 0:1], lw * C * 128,
                                [[128, 128], [128 * 128, NCT], [1, 128]]),
                    )

                # ---- LN1 -> hT, publish (one DMA), AllGather ----
                hT = emit_ln()
                nc.sync.dma_start(
                    out=_ap(hT_loc, 0, [[512, 128], [128 * 512, NCT], [1, 512]]),
                    in_=hT[:],
                )
                hT_full = dram.tile(
                    [NCORES * HTSZ], BF,
                    addr_space="Local" if sim else "Shared", name=f"hT_full{l}",
                )
                if sim:
                    nc.sync.dma_start(
                        out=_ap(hT_full, 0, [[2048, HTSZ // 2048], [1, 2048]]),
                        in_=_ap(hT_loc, 0, [[2048, HTSZ // 2048], [1, 2048]]),
                    )
                else:
                    nc.gpsimd.collective_compute(
                        "AllGather",
                        mybir.AluOpType.bypass,
                        replica_groups=[list(range(NCORES))],
                        ins=[_ap(hT_loc, 0, [[2048, HTSZ // 2048], [1, 2048]])],
                        outs=[_ap(hT_full, 0, [[2048, NCORES * HTSZ // 2048], [1, 2048]])],
                    )

                # gathered h^T lands per rank-chunk so QKV(seq s) starts after
                # 2 chunks, and attention(s) overlaps QKV(s+1)
                hT_sb = [None, None]

                def load_chunk(ch):
                    half = ch // 4
                    if hT_sb[half] is None:
                        hT_sb[half] = big.tile(
                            [128, 4, NCT, 512], BF, tag="big", name=f"hTsb{half}"
                        )
                    nc.sync.dma_start(
                        out=hT_sb[half][:, ch % 4, :, :],
                        in_=_ap(hT_full, ch * HTSZ, [[512, 128], [128 * 512, NCT], [1, 512]]),
                    )

                def htf(ct, ch):
                    return hT_sb[ch // 4][:, ch % 4, ct, :]

                qT_s, kT_s, v_ss, oTm_s = [], [], [], []
                for s in range(SEQ):
                    qT_s.append(qkt.tile([128, 1024], BF, tag=f"qT{s}", name=f"qT{s}"))
                    kT_s.append(qkt.tile([128, 1024], BF, tag=f"kT{s}", name=f"kT{s}"))
                    v_ss.append(vsb_p.tile([128, 8, 2, HD + 1], BF, tag=f"v{s}", name=f"v{s}"))
                    oTm_s.append(otm_p.tile([128, 1024], BF, tag=f"oTm{s}", name=f"oTm{s}"))

                for s in range(SEQ):
                    for hh in range(2):
                        load_chunk(2 * s + hh)
                    # K^T then Q^T for this seq (2 chunks each)
                    for dst, wt_l in ((kT_s[s], wk_t), (qT_s[s], wq_t)):
                        for hh in range(2):
                            ch = 2 * s + hh
                            ps = ps_acc.tile([128, 512], F32, tag="acc", name="acc")
                            for ct in range(NCT):
                                nc.tensor.matmul(
                                    ps[:], wt_l[:, ct, :], htf(ct, ch),
                                    start=(ct == 0), stop=(ct == NCT - 1),
                                )
                            nc.vector.tensor_copy(
                                out=dst[:, hh * 512:(hh + 1) * 512], in_=ps[:]
                            )
                    # V natural [t, my 128 d] with ones column
                    v_sb = v_ss[s]
                    nc.vector.memset(v_sb[:, :, :, HD:HD + 1], 1.0)
                    for i in range(8):
                        tb = s * 8 + i
                        ps = ps_acc.tile([128, 128], F32, tag="acc", name="psv")
                        for ct in range(NCT):
                            nc.tensor.matmul(
                                ps[:], htf(ct, tb // 4)[:, (tb % 4) * 128:(tb % 4 + 1) * 128],
                                wv_t[:, ct, :],
                                start=(ct == 0), stop=(ct == NCT - 1),
                            )
                        nc.vector.tensor_copy(out=v_sb[:, i, 0, 0:HD], in_=ps[:, 0:HD])
                        nc.vector.tensor_copy(out=v_sb[:, i, 1, 0:HD], in_=ps[:, HD:2 * HD])

                    # ---- causal attention: kt-major QK/exp; PV with pT
                    # stationary and V moving (N=65), so output is q-major and
                    # the softmax denominator is a per-partition scalar ----
                    OFF = [128 * (8 * i - i * (i - 1) // 2) for i in range(8)]

                    for hp in range(2):
                        poff = hp * HD
                        pTa = pt_pool.tile([128, 4608], BF, tag="pt", name="pt")

                        onat_all = os_pool.tile([128, 8, HD], BF, tag="onat", name="onat")

                        def emit_pv(qt):
                            ov = ps_ov.tile([128, 128], F32, tag="ov", name="ov")
                            for k2 in range(qt + 1):
                                nc.tensor.matmul(
                                    ov[:, 0:HD + 1],
                                    pTa[:, OFF[k2] + (qt - k2) * 128:OFF[k2] + (qt - k2 + 1) * 128],
                                    v_sb[:, k2, hp, :],
                                    start=(k2 == 0), stop=(k2 == qt),
                                )
                            rec = misc.tile([128, 1], F32, tag="rec", name="rec")
                            nc.vector.reciprocal(rec[:], ov[:, HD:HD + 1])
                            nc.vector.tensor_scalar(
                                out=onat_all[:, qt, :], in0=ov[:, 0:HD], scalar1=rec[:],
                                scalar2=None, op0=mybir.AluOpType.mult,
                            )

                        for kt in range(8):
                            nq = 8 - kt
                            off = 0
                            while off < nq * 128:
                                cols = min(512, nq * 128 - off)
                                st = ps_st.tile([128, 512], F32, tag="st", name="st")
                                nc.tensor.matmul(
                                    st[:, 0:cols],
                                    kT_s[s][poff:poff + HD, kt * 128:(kt + 1) * 128],
                                    qT_s[s][poff:poff + HD, kt * 128 + off:kt * 128 + off + cols],
                                    start=True, stop=True,
                                )
                                nc.scalar.activation(
                                    pTa[:, OFF[kt] + off:OFF[kt] + off + cols], st[:, 0:cols], AF.Exp
                                )
                                off += cols
                            nc.vector.tensor_mul(
                                out=pTa[:, OFF[kt]:OFF[kt] + 128],
                                in0=pTa[:, OFF[kt]:OFF[kt] + 128], in1=mask_t[:],
                            )
                            if kt >= 1:
                                emit_pv(kt - 1)
                        emit_pv(6)
                        emit_pv(7)
                        for qp in range(2):
                            pst = ps_st.tile([128, 512], BF, tag="st", name="ot")
                            for qi in range(4):
                                qt = qp * 4 + qi
                                nc.tensor.transpose(
                                    pst[0:HD, qi * 128:(qi + 1) * 128],
                                    onat_all[:, qt, :], ident[:],
                                )
                            nc.vector.tensor_copy(
                                out=oTm_s[s][poff:poff + HD, qp * 512:(qp + 1) * 512],
                                in_=pst[0:HD, :],
                            )
                    nc.sync.dma_start(
                        out=_ap(o_loc, 2 * s * 128 * 512, [[512, 128], [128 * 512, 2], [1, 512]]),
                        in_=oTm_s[s][:],
                    )
                o_recv = dram.tile([NCORES * 128 * 512], BF, name=f"orecv{l}")
                if sim:
                    nc.sync.dma_start(
                        out=_ap(o_recv, 0, [[2048, NCORES * 128 * 512 // 2048], [1, 2048]]),
                        in_=_ap(o_loc, 0, [[2048, NCORES * 128 * 512 // 2048], [1, 2048]]),
                    )
                else:
                    nc.gpsimd.collective_compute(
                        "AllToAll",
                        mybir.AluOpType.bypass,
                        replica_groups=[list(range(NCORES))],
                        ins=[_ap(o_loc, 0, [[2048, NCORES * 128 * 512 // 2048], [1, 2048]])],
                        outs=[_ap(o_recv, 0, [[2048, NCORES * 128 * 512 // 2048], [1, 2048]])],
                    )
                orv = [otm_p.tile([128, 4, 512], BF, tag=f"orv{h_}", name="orv")
                       for h_ in range(2)]
                for h_ in range(2):
                    nc.sync.dma_start(
                        out=orv[h_][:],
                        in_=_ap(o_recv, h_ * 4 * 128 * 512,
                                [[512, 128], [128 * 512, 4], [1, 512]]),
                    )
                wo_ts = []
                for nf in range(2):
                    for cg in range(2):
                        wt = w4.tile([128, 4, 512], BF, tag="w", name="wot")
                        nc.sync.dma_start(
                            out=wt[:],
                            in_=_ap(wo_d[0:1, 0:1, 0:1],
                                    lw * C * C + cg * 512 * C + nf * 512,
                                    [[C, 128], [128 * C, 4], [1, 512]]),
                        )
                        wo_ts.append(wt)
                # tt-outer Wo + residual, LN2 stats fused per tile
                h2_tiles = []
                for tt in range(NT):
                    for nf in range(2):
                        ps = ps_acc.tile([128, 512], F32, tag="acc", name="acc")
                        for cg in range(2):
                            wt = wo_ts[nf * 2 + cg]
                            for ci in range(4):
                                nc.tensor.matmul(
                                    ps[:], orv[cg][:, ci, tt * 128:(tt + 1) * 128], wt[:, ci, :],
                                    start=(cg == 0 and ci == 0), stop=False,
                                )
                        nc.tensor.matmul(
                            ps[:], ones1[:], bo_r[:, nf * 512:(nf + 1) * 512],
                            start=False, stop=True,
                        )
                        xs = x_t[tt][:, nf * 512:(nf + 1) * 512]
                        nc.vector.tensor_add(out=xs, in0=xs, in1=ps[:])
                    stats = misc.tile([128, 2, 6], F32, name="stats", tag="stats")
                    xv = x_t[tt][:].rearrange("p (s d) -> p s d", s=2)
                    nc.vector.bn_stats(out=stats[:, 0, :], in_=xv[:, 0, :])
                    nc.vector.bn_stats(out=stats[:, 1, :], in_=xv[:, 1, :])
                    mv = misc.tile([128, 2], F32, name="mv", tag="mv")
                    nc.vector.bn_aggr(out=mv[:], in_=stats[:])
                    rstd = misc.tile([128, 1], F32, name="rstd", tag="rstd")
                    nc.scalar.activation(rstd[:], mv[:, 1:2], AF.Sqrt, bias=eps_t[:])
                    nc.vector.reciprocal(rstd[:], rstd[:])
                    h = hpool.tile([128, C], BF, tag="h", name="h")
                    nc.vector.tensor_scalar(
                        out=h[:], in0=x_t[tt][:], scalar1=mv[:, 0:1], scalar2=rstd[:],
                        op0=mybir.AluOpType.subtract, op1=mybir.AluOpType.mult,
                    )
                    h2_tiles.append(h)
                h2T = tpool.tile([128, NCT, 512], BF, tag="hTall", name="h2Tall")
                for ct in range(NCT):
                    pst = ps_st.tile([128, 512], BF, tag="st", name="pst")
                    for tt in range(NT):
                        nc.tensor.transpose(
                            pst[:, tt * 128:(tt + 1) * 128],
                            h2_tiles[tt][:, ct * 128:(ct + 1) * 128],
                            ident[:],
                        )
                    nc.vector.tensor_copy(out=h2T[:, ct, :], in_=pst[:])

                # ---- FFN (token-parallel, biases in-psum / in-activation) ----
                ug = big.tile([128, NFT, 512], BF, tag="big", name="ug")
                for fg in range(8):
                    pss = [ps_acc.tile([128, 512], F32, tag="acc", name="acc") for _ in range(4)]
                    for hf in range(2):
                        wt = w4.tile([128, 4, 512], BF, tag="w", name="w1t")
                        nc.sync.dma_start(
                            out=wt[:],
                            in_=_ap(w1_d[0:1, 0:1, 0:1],
                                    lw * C * FF + hf * 512 * FF + fg * 512,
                                    [[FF, 128], [128 * FF, 4], [1, 512]]),
                        )
                        for ci in range(4):
                            ct = hf * 4 + ci
                            for f4 in range(4):
                                nc.tensor.matmul(
                                    pss[f4][:], wt[:, ci, f4 * 128:(f4 + 1) * 128], h2T[:, ct, :],
                                    start=(ct == 0), stop=(ct == NCT - 1),
                                )
                    for f4 in range(4):
                        ft = fg * 4 + f4
                        nc.scalar.activation(
                            ug[:, ft, :], pss[f4][:], AF.Gelu, bias=b1_t[:, ft:ft + 1]
                        )
                for nf in range(2):
                    pss = [ps_acc.tile([128, 512], F32, tag="acc", name="acc") for _ in range(4)]
                    for g8 in range(8):
                        wt = w4.tile([128, 4, 512], BF, tag="w", name="w2t")
                        nc.sync.dma_start(
                            out=wt[:],
                            in_=_ap(w2_d[0:1, 0:1, 0:1],
                                    lw * FF * C + g8 * 512 * C + nf * 512,
                                    [[C, 128], [128 * C, 4], [1, 512]]),
                        )
                        for fi in range(4):
                            ft = g8 * 4 + fi
                            for tt in range(NT):
                                nc.tensor.matmul(
                                    pss[tt][:], ug[:, ft, tt * 128:(tt + 1) * 128], wt[:, fi, :],
                                    start=(ft == 0), stop=False,
                                )
                    for tt in range(NT):
                        nc.tensor.matmul(
                            pss[tt][:], ones1[:], b2_r[:, nf * 512:(nf + 1) * 512],
                            start=False, stop=True,
                        )
                        xs = x_t[tt][:, nf * 512:(nf + 1) * 512]
                        nc.vector.tensor_add(out=xs, in0=xs, in1=pss[tt][:])

            # ---- final LN (folded) + token-sharded lm_head ----
            hfT = emit_ln()
            for ch in range(NCH):
                wlm_c = big.tile([128, NCT, VCW], BF, tag="big", name="wlm_c")
                nc.sync.dma_start(
                    out=wlm_c[:],
                    in_=_ap(wlm_d[0:1, 0:1, 0:1, 0:1], ch * NCT * 128 * VCW,
                            [[VCW, 128], [128 * VCW, NCT], [1, VCW]]),
                )
                for th in range(2):
                    lg = lgout.tile([128, 2, VCW], BF, tag="lg", name="lg")
                    for ti in range(2):
                        tt = th * 2 + ti
                        ps = ps_acc.tile([128, VCW], F32, tag="acc", name="acc")
                        for ct in range(NCT):
                            nc.tensor.matmul(
                                ps[:], hfT[:, ct, tt * 128:(tt + 1) * 128], wlm_c[:, ct, :],
                                start=(ct == 0), stop=(ct == NCT - 1),
                            )
                        if ti % 2 == 0:
                            nc.vector.tensor_copy(out=lg[:, ti, :], in_=ps[:])
                        else:
                            nc.scalar.activation(lg[:, ti, :], ps[:], AF.Copy)
                    nc.sync.dma_start(
                        out=_ap(logits_d[0:1, 0:1], th * 2 * 128 * V + ch * VCW,
                                [[V, 128], [128 * V, 2], [1, VCW]]),
                        in_=lg[:],
                    )

    nc.compile()
    _prog_cache[key] = nc
    return nc


def _prep_inputs(inputs):
    f = {k: np.asarray(v) for k, v in inputs.items()}
    idx = f["idx"].astype(np.int64)
    emb = f["emb"].astype(np.float32)
    pos = f["pos_enc"].astype(np.float32)
    x_full = emb[idx] + pos[None, :, :]          # [B,T,C] f32

    scale = HD ** -0.5
    g1 = f["ln1_g"].astype(np.float32)
    b1ln = f["ln1_b"].astype(np.float32)
    g2 = f["ln2_g"].astype(np.float32)
    b2ln = f["ln2_b"].astype(np.float32)
    gf = f["lnf_g"].astype(np.float32)
    bfln = f["lnf_b"].astype(np.float32)
    W1 = f["W1"].astype(np.float32)
    Wv = f["Wv"].astype(np.float32)
    Wo = f["Wo"].astype(np.float32)
    Wlm = f["Wlm"].astype(np.float32)

    bf = lambda a: np.ascontiguousarray(a, dtype=np.float32).astype(BF16NP)
    b1_f = f["b1"].astype(np.float32) + np.einsum("lc,lcf->lf", b2ln, W1)
    bo_f = f["bo"].astype(np.float32) + np.einsum(
        "ld,ldc->lc", np.einsum("lc,lcd->ld", b1ln, Wv), Wo
    )
    blm_f = f["blm"].astype(np.float32) + bfln @ Wlm

    wq_s = f["Wq"].astype(np.float32) * scale * g1[:, :, None]
    wk_s = f["Wk"].astype(np.float32) * g1[:, :, None]
    wv_s = Wv * g1[:, :, None]
    wlm_blocks = np.ascontiguousarray(
        (Wlm * gf[:, None]).reshape(NCT, 128, NCH, VCW).transpose(2, 0, 1, 3)
    ).astype(BF16NP)

    shared = {
        "w1": bf(W1 * g2[:, :, None]),
        "w2": bf(f["W2"]),
        "b1": b1_f.astype(np.float32),
        "bo": bf(bo_f),
        "wo": bf(Wo),
        "b2": bf(f["b2"]),
        "wlm": wlm_blocks,
        "maskd": np.triu(np.ones((128, 128), dtype=np.float32)).astype(BF16NP),
    }

    x_flat = np.ascontiguousarray(x_full.reshape(B * T, C), dtype=np.float32)
    in_maps = []
    for c in range(NCORES):
        hc = slice(c * 128, (c + 1) * 128)
        im = dict(shared)
        # core c owns flat tokens [c*512, (c+1)*512) — seq c//2, half c%2 —
        # so gathered-hT chunk r is exactly rank r's contiguous token block
        im["x0"] = x_flat[c * TL:(c + 1) * TL]
        im["wq"] = bf(wq_s[:, :, hc])
        im["wk"] = bf(wk_s[:, :, hc])
        im["wv"] = bf(wv_s[:, :, hc])
        in_maps.append(im)
    return in_maps, blm_f


def kernel(**inputs):
    nc = _build()
    in_maps, blm_f = _prep_inputs(inputs)
    res = run_bass_kernel_spmd(nc, in_maps, list(range(NCORES)))
    full = np.zeros((B * T, V), dtype=np.float32)
    for c in range(NCORES):
        full[c * TL:(c + 1) * TL, :] = np.asarray(
            res.results[c]["logits"], dtype=np.float32
        )
    full += blm_f[None, :]
    return full.reshape(B, T, V)
